# revision 27
# baseline (speedup 1.0000x reference)
"""Trainium2 Bass kernel for CrossRegionRelationalReasoning (gnn_message_passing).

Computation: mean-pool [B,N,C,4,4] -> [B,N,C], project to HID, then 3 layers of
tiny self-attention (N=5 regions, 4 heads) with residual + LayerNorm.
Returns (x, attn) like the reference.

Sharding: pure data parallel. B=4096 split as 512 batches per core across the
8 NeuronCores; the ~1MB weight set is replicated.

Per-core design:
- token tiles are (region n, batch-block of 128): partitions = batch, so
  attention (which needs all 5 regions of one batch on one partition) works
  directly with strided/broadcast APs on the free dim.
- 16:1 spatial mean pooling: DMA casts fp32->bf16 inline (SWDGE), then a
  binary add-tree on the vector engine (bf16 2x mode); the /16 is folded
  into w_in host-side.
- All matmuls on PE in bf16 (weights host-cast), accumulating fp32 in PSUM.
  Biases are added via rank-1 ones x bias matmuls, the residual via an
  identity matmul, so PSUM accumulates the whole pre-LN activation.
- LayerNorm stats ride along the PSUM-evacuation passes on the scalar engine
  (activation accum_out); rstd = rsqrt via DVE bit-trick + Newton so ScalarE
  stays on one activation-table set (no table-swap stalls).
- Work is emitted software-pipelined across batch blocks (A = proj+attention,
  B = LN tail) so each engine's in-order stream has no long dependency
  stalls: while one block's O-projection runs on PE/ACT, the next block's
  attention keeps the vector engine busy.
"""

import sys

sys.path.insert(0, "/opt/trn_rl_repo")

import numpy as np
import ml_dtypes

B, N, C, H, W = 4096, 5, 512, 4, 4
HW = H * W
HID, HEADS, LAYERS = 256, 4, 3
HD = HID // HEADS
NCORES = 8
P = 128

_BUILD_CACHE = {}


def _build(flags, BC):
    """Build the per-core Bass program. flags = (has_mask, has_b_in, has_b_qkv,
    has_b_o, has_ln_affine)."""
    import concourse.bass as bass
    import concourse.bacc as bacc
    import concourse.mybir as mybir
    import concourse.tile as tile
    from concourse.masks import make_identity

    has_mask, has_b_in, has_b_qkv, has_b_o, has_ln = flags
    f32 = mybir.dt.float32
    bf16 = mybir.dt.bfloat16
    i32 = mybir.dt.int32
    Alu = mybir.AluOpType
    Act = mybir.ActivationFunctionType

    NBLK = BC // P
    CH = C * HW  # 8192
    HNM = HEADS * N * N  # 100

    nc = bacc.Bacc("TRN2", target_bir_lowering=False, debug=False,
                   num_devices=NCORES)

    rf = nc.dram_tensor("rf", [BC, N, CH], f32, kind="ExternalInput")
    w_in_d = nc.dram_tensor("w_in_t", [4, P, HID], bf16, kind="ExternalInput")
    wqkv_d = nc.dram_tensor("wqkv_t", [LAYERS, 2, P, 3 * HID], bf16,
                            kind="ExternalInput")
    wo_d = nc.dram_tensor("wo_t", [LAYERS, 2, P, HID], bf16,
                          kind="ExternalInput")
    if has_b_in:
        b_in_d = nc.dram_tensor("b_in_t", [1, HID], bf16, kind="ExternalInput")
    if has_b_qkv:
        bqkv_d = nc.dram_tensor("bqkv_t", [LAYERS, 3 * HID], bf16,
                                kind="ExternalInput")
    if has_b_o:
        bo_d = nc.dram_tensor("bo_t", [LAYERS, HID], bf16, kind="ExternalInput")
    if has_ln:
        ln_g_d = nc.dram_tensor("ln_g_t", [LAYERS, HID], f32,
                                kind="ExternalInput")
        ln_b_d = nc.dram_tensor("ln_b_t", [LAYERS, HID], f32,
                                kind="ExternalInput")
    if has_mask:
        mask_d = nc.dram_tensor("mask_t", [N * N], f32, kind="ExternalInput")

    x_out = nc.dram_tensor("x_out", [BC, N, HID], f32, kind="ExternalOutput")
    attn_out = nc.dram_tensor("attn_out", [BC, HEADS * N * N], f32,
                              kind="ExternalOutput")

    def bcast_ap(base, off, dims):
        # custom AP over a tile: keep partition dim, replace free dims
        return bass.AP(tensor=base.tensor, offset=base.offset + off,
                       ap=[list(base.ap[0])] + [list(d) for d in dims])

    with tile.TileContext(nc) as tc:
        with (
            tc.tile_pool(name="consts", bufs=1) as consts,
            tc.tile_pool(name="xr", bufs=2) as xr_pool,
            tc.tile_pool(name="tree", bufs=1) as tree_pool,
            tc.tile_pool(name="pooled", bufs=2) as pooled_pool,
            tc.tile_pool(name="small", bufs=2) as small_pool,
            tc.tile_pool(name="xfer", bufs=6) as xfer_pool,
            tc.tile_pool(name="x", bufs=6) as x_pool,
            tc.tile_pool(name="qkv", bufs=3) as qkv_pool,
            # DVE-produced, DVE-consumed intermediates: bufs=1 (DVE is
            # serial anyway); cross-engine tiles get bufs>=2
            tc.tile_pool(name="attnd", bufs=1) as attnd_pool,
            tc.tile_pool(name="attnw", bufs=3) as attnw_pool,
            tc.tile_pool(name="stats", bufs=3) as stats_pool,
            tc.tile_pool(name="y", bufs=3) as y_pool,
            tc.tile_pool(name="psA", bufs=2, space="PSUM") as psA,
            tc.tile_pool(name="psT", bufs=1, space="PSUM") as psT,
            tc.tile_pool(name="psC", bufs=1, space="PSUM") as psC,
        ):
            # ---- constants ----
            ident32 = consts.tile([P, P], f32)
            make_identity(nc, ident32[:])
            identbf = consts.tile([P, P], bf16)
            make_identity(nc, identbf[:])
            magic_sb = consts.tile([P, N], i32)
            nc.vector.memset(magic_sb[:], 0x5F3759DF)

            w_in_sb = consts.tile([P, 4, HID], bf16)
            nc.sync.dma_start(out=w_in_sb[:],
                              in_=w_in_d[:].rearrange("k p o -> p k o"))
            wqkv_sb = consts.tile([P, LAYERS, 2, 3 * HID], bf16)
            nc.sync.dma_start(out=wqkv_sb[:],
                              in_=wqkv_d[:].rearrange("l k p o -> p l k o"))
            wo_sb = consts.tile([P, LAYERS, 2, HID], bf16)
            nc.sync.dma_start(out=wo_sb[:],
                              in_=wo_d[:].rearrange("l k p o -> p l k o"))

            if has_b_in or has_b_qkv or has_b_o:
                ones_row = consts.tile([1, P], bf16)
                nc.vector.memset(ones_row[:], 1.0)
            if has_b_in:
                b_in_sb = consts.tile([1, HID], bf16)
                nc.sync.dma_start(out=b_in_sb[:], in_=b_in_d[:])
            if has_b_qkv:
                bqkv_sb = consts.tile([1, LAYERS, 3 * HID], bf16)
                nc.sync.dma_start(out=bqkv_sb[:],
                                  in_=bqkv_d[:].rearrange("l o -> 1 l o"))
            if has_b_o:
                bo_sb = consts.tile([1, LAYERS, HID], bf16)
                nc.sync.dma_start(out=bo_sb[:],
                                  in_=bo_d[:].rearrange("l o -> 1 l o"))
            if has_ln:
                g_rep = consts.tile([P, LAYERS, HID], f32)
                nc.gpsimd.dma_start(
                    out=g_rep[:],
                    in_=bass.AP(tensor=ln_g_d, offset=0,
                                ap=[[0, P], [HID, LAYERS], [1, HID]]))
                b_rep = consts.tile([P, LAYERS, HID], f32)
                nc.gpsimd.dma_start(
                    out=b_rep[:],
                    in_=bass.AP(tensor=ln_b_d, offset=0,
                                ap=[[0, P], [HID, LAYERS], [1, HID]]))
            if has_mask:
                mask_rep = consts.tile([P, N * N], f32)
                nc.gpsimd.dma_start(
                    out=mask_rep[:],
                    in_=bass.AP(tensor=mask_d, offset=0,
                                ap=[[0, P], [1, N * N]]))

            xs = [None] * NBLK
            st = {}  # (l, blk) -> (y, sums, sumsq) handed from A to B

            def emit_p1(blk):
                """pooling + input projection -> xs[blk]"""
                b0 = blk * P
                x = x_pool.tile([P, N, HID], bf16, tag="x")
                xs[blk] = x
                for n in range(N):
                    xr = xr_pool.tile([P, C, HW], bf16)
                    nc.gpsimd.dma_start(
                        out=xr[:],
                        in_=rf[b0:b0 + P, n, :].rearrange("p (c h) -> p c h",
                                                          h=HW))
                    t1 = tree_pool.tile([P, C, 8], bf16, tag="t1")
                    nc.vector.tensor_add(t1[:], xr[:, :, 0:8], xr[:, :, 8:16])
                    t2 = tree_pool.tile([P, C, 4], bf16, tag="t2")
                    nc.vector.tensor_add(t2[:], t1[:, :, 0:4], t1[:, :, 4:8])
                    t3 = tree_pool.tile([P, C, 2], bf16, tag="t3")
                    nc.vector.tensor_add(t3[:], t2[:, :, 0:2], t2[:, :, 2:4])
                    pooled = pooled_pool.tile([P, C], f32, tag="pooled")
                    nc.vector.tensor_add(pooled[:], t3[:, :, 0], t3[:, :, 1])

                    pooledT_ps = psA.tile([P, 4, P], f32, tag="psA")
                    for j in range(4):
                        nc.tensor.transpose(pooledT_ps[:, j, :],
                                            pooled[:, j * P:(j + 1) * P],
                                            ident32[:])
                    pooledT = pooled_pool.tile([P, 4, P], bf16, tag="pooledT")
                    nc.scalar.copy(pooledT[:], pooledT_ps[:])

                    x_ps = psA.tile([P, HID], f32, tag="psA")
                    for j in range(4):
                        nc.tensor.matmul(x_ps[:], pooledT[:, j, :],
                                         w_in_sb[:, j, :], start=(j == 0),
                                         stop=(j == 3 and not has_b_in))
                    if has_b_in:
                        nc.tensor.matmul(x_ps[:], ones_row[:], b_in_sb[:],
                                         start=False, stop=True)
                    nc.scalar.copy(x[:, n, :], x_ps[:])

            def emit_A1(l, blk):
                """x transposes + QKV projection -> qkv"""
                x = xs[blk]
                qkv = qkv_pool.tile([P, N, 3, HID], bf16, tag="qkv")
                # transposes + evacuations first, then matmuls: keeps the
                # in-order ACT stream free of PE round-trip stalls
                xTs = []
                for n in range(N):
                    xT_ps = psT.tile([P, 2, P], bf16, tag="psT")
                    for j in range(2):
                        nc.tensor.transpose(xT_ps[:, j, :],
                                            x[:, n, j * P:(j + 1) * P],
                                            identbf[:])
                    xT = xfer_pool.tile([P, 2, P], bf16, tag="xT")
                    nc.scalar.copy(xT[:], xT_ps[:])
                    xTs.append(xT)
                for n in range(N):
                    xT = xTs[n]
                    qkv_ps = psA.tile([P, 3 * HID], f32, tag="psA")
                    for c0, c1 in ((0, 512), (512, 768)):
                        for j in range(2):
                            nc.tensor.matmul(qkv_ps[:, c0:c1], xT[:, j, :],
                                             wqkv_sb[:, l, j, c0:c1],
                                             start=(j == 0),
                                             stop=(j == 1 and not has_b_qkv))
                        if has_b_qkv:
                            nc.tensor.matmul(qkv_ps[:, c0:c1], ones_row[:],
                                             bqkv_sb[:, l, c0:c1],
                                             start=False, stop=True)
                    nc.scalar.copy(qkv[:, n, :, :], qkv_ps[:])
                st[("qkv", l, blk)] = qkv

            def emit_A2(l, blk):
                """attention (batch on partitions) -> ctx"""
                b0 = blk * P
                qkv = st.pop(("qkv", l, blk))
                # qkv free strides: n:768, slot:256, h:64, d:1
                qb = qkv[:]
                prod = attnd_pool.tile([P, HNM, HD], bf16, tag="bigprod")
                for h in range(HEADS):
                    # out block (n, m, d) at rows h*25..h*25+25
                    nc.vector.tensor_mul(
                        bcast_ap(prod[:], h * N * N * HD,
                                 [[N * HD, N], [HD, N], [1, HD]]),
                        bcast_ap(qb, h * HD,
                                 [[3 * HID, N], [0, N], [1, HD]]),
                        bcast_ap(qb, HID + h * HD,
                                 [[0, N], [3 * HID, N], [1, HD]]))
                s1 = attnd_pool.tile([P, HNM, 32], bf16, tag="s1")
                nc.vector.tensor_add(s1[:], prod[:, :, 0:32], prod[:, :, 32:64])
                s2 = attnd_pool.tile([P, HNM, 16], bf16, tag="s2")
                nc.vector.tensor_add(s2[:], s1[:, :, 0:16], s1[:, :, 16:32])
                s3 = attnd_pool.tile([P, HNM, 8], bf16, tag="s3")
                nc.vector.tensor_add(s3[:], s2[:, :, 0:8], s2[:, :, 8:16])
                s4 = attnd_pool.tile([P, HNM, 4], bf16, tag="s4")
                nc.vector.tensor_add(s4[:], s3[:, :, 0:4], s3[:, :, 4:8])
                s5 = attnd_pool.tile([P, HNM, 2], bf16, tag="s5")
                nc.vector.tensor_add(s5[:], s4[:, :, 0:2], s4[:, :, 2:4])
                scores = attnw_pool.tile([P, HEADS, N, N], bf16, tag="scores")
                nc.vector.tensor_add(
                    scores[:].rearrange("p h n m -> p (h n m)"),
                    s5[:, :, 0], s5[:, :, 1])
                if has_mask:
                    nc.vector.tensor_add(
                        scores[:], scores[:],
                        bcast_ap(mask_rep[:], 0, [[0, HEADS], [1, N * N]]))
                exps = attnw_pool.tile([P, HEADS, N, N], bf16, tag="exps")
                nc.scalar.activation(exps[:], scores[:], Act.Exp,
                                     scale=1.0 / np.sqrt(HD))
                Z = stats_pool.tile([P, HEADS * N], f32, tag="Z")
                nc.vector.tensor_reduce(
                    Z[:], exps[:].rearrange("p h n m -> p (h n) m"),
                    axis=mybir.AxisListType.X, op=Alu.add)
                rZ = stats_pool.tile([P, HEADS * N], f32, tag="rZ")
                nc.vector.reciprocal(rZ[:], Z[:])
                attnb = attnd_pool.tile([P, HEADS, N, N], bf16, tag="attnb")
                rz_ap = bcast_ap(rZ[:], 0, [[N, HEADS], [1, N], [0, N]])
                nc.vector.tensor_mul(attnb[:], exps[:], rz_ap)
                if l == LAYERS - 1:
                    attnf = attnw_pool.tile([P, HEADS, N, N], f32, tag="attnf")
                    nc.vector.tensor_mul(attnf[:], exps[:], rz_ap)
                    nc.sync.dma_start(
                        out=attn_out[b0:b0 + P, :],
                        in_=attnf[:].rearrange("p h n m -> p (h n m)"))

                # ctx[b, n, h, d] = sum_m attn[b,h,n,m] * V[b,m,h,d]
                # A2 = attn duplicated in d-pairs so products keep a step-1
                # even innermost dim (DVE 2x mode)
                A2 = attnd_pool.tile([P, HNM, 2], bf16, tag="A2")
                nc.vector.tensor_copy(
                    A2[:], bcast_ap(attnb[:], 0, [[1, HNM], [0, 2]]))
                # prod2 layout (n, h, m, d): strides n:1280, h:320, m:64, d:1
                prod2 = attnd_pool.tile([P, N * HEADS, N, HD], bf16,
                                        tag="bigprod")
                p2 = prod2[:]
                for h in range(HEADS):
                    for m in range(N):
                        nc.vector.tensor_mul(
                            bcast_ap(p2, h * N * HD + m * HD,
                                     [[HEADS * N * HD, N], [2, HD // 2],
                                      [1, 2]]),
                            bcast_ap(A2[:], (h * N * N + m) * 2,
                                     [[2 * N, N], [0, HD // 2], [1, 2]]),
                            bcast_ap(qb, 2 * HID + m * 3 * HID + h * HD,
                                     [[0, N], [2, HD // 2], [1, 2]]))
                # m-tree, d stays innermost -> all adds run 2x
                c1 = attnd_pool.tile([P, N * HEADS, 2, HD], bf16, tag="c1")
                nc.vector.tensor_add(
                    c1[:],
                    bcast_ap(p2, 0, [[N * HD, N * HEADS], [2 * HD, 2],
                                     [1, HD]]),
                    bcast_ap(p2, HD, [[N * HD, N * HEADS], [2 * HD, 2],
                                      [1, HD]]))
                c2 = attnd_pool.tile([P, N * HEADS, HD], bf16, tag="c2")
                nc.vector.tensor_add(c2[:], c1[:, :, 0, :], c1[:, :, 1, :])
                ctx = attnw_pool.tile([P, N, HID], bf16, tag="ctx")
                nc.vector.tensor_add(
                    ctx[:].rearrange("p n (h d) -> p (n h) d", d=HD),
                    c2[:],
                    bcast_ap(p2, 4 * HD, [[N * HD, N * HEADS], [1, HD]]))

                st[("ctx", l, blk)] = ctx

            def emit_A3(l, blk):
                """ctx transposes + O-projection + residual -> y"""
                x = xs[blk]
                ctx = st.pop(("ctx", l, blk))
                ctxTs = []
                for n in range(N):
                    ctxT_ps = psT.tile([P, 2, P], bf16, tag="psT")
                    for j in range(2):
                        nc.tensor.transpose(ctxT_ps[:, j, :],
                                            ctx[:, n, j * P:(j + 1) * P],
                                            identbf[:])
                    ctxT = xfer_pool.tile([P, 2, P], bf16, tag="ctxT")
                    nc.scalar.copy(ctxT[:], ctxT_ps[:])
                    ctxTs.append(ctxT)
                o_big = psC.tile([P, N, HID], f32, tag="psC")
                for n in range(N):
                    ctxT = ctxTs[n]
                    for j in range(2):
                        nc.tensor.matmul(o_big[:, n, :], ctxT[:, j, :],
                                         wo_sb[:, l, j, :],
                                         start=(j == 0), stop=False)
                    if has_b_o:
                        nc.tensor.matmul(o_big[:, n, :], ones_row[:],
                                         bo_sb[:, l, :], start=False,
                                         stop=False)
                    nc.tensor.matmul(o_big[:, n, :], identbf[:], x[:, n, :],
                                     start=False, stop=True)
                # one batched evacuation frees the PSUM slot early and lets
                # the LN tail read SBUF (faster DVE modes)
                y = y_pool.tile([P, N, HID], f32, tag="y")
                nc.scalar.copy(y[:], o_big[:])
                st[("y", l, blk)] = y

            def emit_B(l, blk):
                """LayerNorm tail -> xs[blk] (and final output DMA)"""
                b0 = blk * P
                y = st.pop(("y", l, blk))
                # per-token mean/variance on DVE
                mv = stats_pool.tile([P, N, 2], f32, tag="mv")
                for n in range(N):
                    bns = stats_pool.tile([P, 6], f32, tag="bns")
                    nc.vector.bn_stats(bns[:], y[:, n, :])
                    nc.vector.bn_aggr(mv[:, n, :], bns[:])
                var = stats_pool.tile([P, N], f32, tag="var")
                nc.vector.tensor_scalar_add(var[:], mv[:, :, 1], 1e-5)
                # rstd = 1/sqrt(var) on DVE: bit-trick + 3 Newton steps
                sh = stats_pool.tile([P, N], i32, tag="sh")
                nc.vector.tensor_scalar(sh[:], var[:].bitcast(i32), 1, None,
                                        op0=Alu.logical_shift_right)
                rstd = stats_pool.tile([P, N], f32, tag="rstd")
                nc.vector.tensor_sub(rstd[:].bitcast(i32), magic_sb[:], sh[:])
                for _ in range(3):
                    nt = stats_pool.tile([P, N], f32, tag="nt")
                    nc.vector.tensor_mul(nt[:], rstd[:], rstd[:])
                    nc.vector.tensor_mul(nt[:], nt[:], var[:])
                    nc.vector.tensor_scalar(nt[:], nt[:], -0.5, 1.5,
                                            op0=Alu.mult, op1=Alu.add)
                    nc.vector.tensor_mul(rstd[:], rstd[:], nt[:])

                final = l == LAYERS - 1
                if final:
                    # last layer's x goes straight to HBM: keep fp32
                    xn = x_pool.tile([P, N, HID], f32, tag="xf")
                else:
                    xn = x_pool.tile([P, N, HID], bf16, tag="x")
                for n in range(N):
                    nc.vector.tensor_scalar(xn[:, n, :], y[:, n, :],
                                            mv[:, n, 0:1], rstd[:, n:n + 1],
                                            op0=Alu.subtract, op1=Alu.mult)
                    if has_ln:
                        nc.vector.tensor_mul(xn[:, n, :], xn[:, n, :],
                                             g_rep[:, l, :])
                        nc.vector.tensor_add(xn[:, n, :], xn[:, n, :],
                                             b_rep[:, l, :])
                xs[blk] = xn
                if l == LAYERS - 1:
                    nc.sync.dma_start(out=x_out[b0:b0 + P, :, :], in_=xn[:])

            # ---- software-pipelined emission ----
            # Slot k handles A-stage (l,blk) = S[k]. Emitting next slot's QKV
            # (A1) before this slot's O-projection (A3), and LayerNorm (B)
            # two slots late, keeps every engine's in-order stream stall-free:
            # PE/ACT always have independent work while DVE runs attention.
            if NBLK == 1:
                emit_p1(0)
                for l in range(LAYERS):
                    emit_A1(l, 0)
                    emit_A2(l, 0)
                    emit_A3(l, 0)
                    emit_B(l, 0)
            else:
                S = [(l, blk) for l in range(LAYERS) for blk in range(NBLK)]
                emit_p1(0)
                emit_p1(1)
                for k in range(len(S)):
                    if k >= 2:
                        emit_B(*S[k - 2])
                    emit_A1(*S[k])
                    if k + 2 < NBLK:
                        emit_p1(k + 2)
                    if k >= 1:
                        emit_A3(*S[k - 1])
                    emit_A2(*S[k])
                emit_A3(*S[-1])
                emit_B(*S[-2])
                emit_B(*S[-1])

    nc.compile()
    return nc


def _get_program(flags, BC):
    key = (flags, BC)
    if key not in _BUILD_CACHE:
        _BUILD_CACHE[key] = _build(flags, BC)
    return _BUILD_CACHE[key]


def kernel(region_features, adjacency, w_in, b_in, wq, bq, wk, bk, wv, bv,
           wo, bo, ln_g, ln_b, _trace=False, _bc=None):
    from concourse.bass_utils import run_bass_kernel_spmd

    region_features = np.asarray(region_features)
    adjacency = np.asarray(adjacency, dtype=np.float32)
    w_in = np.asarray(w_in, dtype=np.float32)
    b_in = np.asarray(b_in, dtype=np.float32)
    wq, wk, wv, wo = (np.asarray(a, dtype=np.float32) for a in (wq, wk, wv, wo))
    bq, bk, bv, bo = (np.asarray(a, dtype=np.float32) for a in (bq, bk, bv, bo))
    ln_g = np.asarray(ln_g, dtype=np.float32)
    ln_b = np.asarray(ln_b, dtype=np.float32)

    Btot = region_features.shape[0]
    BC = _bc if _bc is not None else Btot // NCORES
    assert Btot == BC * NCORES

    has_mask = bool((adjacency == 0).any())
    has_b_in = bool(np.any(b_in != 0))
    has_b_qkv = bool(np.any(bq != 0) or np.any(bk != 0) or np.any(bv != 0))
    has_b_o = bool(np.any(bo != 0))
    has_ln = bool(np.any(ln_g != 1) or np.any(ln_b != 0))
    flags = (has_mask, has_b_in, has_b_qkv, has_b_o, has_ln)

    nc = _get_program(flags, BC)

    bf = ml_dtypes.bfloat16
    # host-side weight prep: fold /16 pooling into w_in, chunk contraction dim
    w_in_t = np.ascontiguousarray((w_in / HW).reshape(4, P, HID)).astype(bf)
    wqkv = np.concatenate([wq, wk, wv], axis=2)  # [L, 256, 768]
    wqkv_t = np.ascontiguousarray(wqkv.reshape(LAYERS, 2, P, 3 * HID)).astype(bf)
    wo_t = np.ascontiguousarray(wo.reshape(LAYERS, 2, P, HID)).astype(bf)

    base_map = {
        "w_in_t": w_in_t,
        "wqkv_t": wqkv_t,
        "wo_t": wo_t,
    }
    if has_b_in:
        base_map["b_in_t"] = b_in.reshape(1, HID).astype(bf)
    if has_b_qkv:
        base_map["bqkv_t"] = np.concatenate([bq, bk, bv], axis=1).astype(bf)
    if has_b_o:
        base_map["bo_t"] = bo.astype(bf)
    if has_ln:
        base_map["ln_g_t"] = ln_g
        base_map["ln_b_t"] = ln_b
    if has_mask:
        base_map["mask_t"] = np.where(adjacency == 0, -1e9,
                                      0.0).astype(np.float32).reshape(N * N)

    rf_flat = np.ascontiguousarray(
        region_features.reshape(Btot, N, C * HW)).astype(np.float32)
    in_maps = []
    for c in range(NCORES):
        m = dict(base_map)
        m["rf"] = rf_flat[c * BC:(c + 1) * BC]
        in_maps.append(m)

    res = run_bass_kernel_spmd(nc, in_maps, core_ids=list(range(NCORES)),
                               trace=_trace)
    kernel.last_results = res

    x_full = np.concatenate([r["x_out"] for r in res.results], axis=0)
    attn_full = np.concatenate(
        [r["attn_out"].reshape(BC, HEADS, N, N) for r in res.results], axis=0)
    return x_full, attn_full


# revision 28
# speedup vs baseline: 1.0086x; 1.0086x over previous
"""Trainium2 Bass kernel for CrossRegionRelationalReasoning (gnn_message_passing).

Computation: mean-pool [B,N,C,4,4] -> [B,N,C], project to HID, then 3 layers of
tiny self-attention (N=5 regions, 4 heads) with residual + LayerNorm.
Returns (x, attn) like the reference.

Sharding: pure data parallel. B=4096 split as 512 batches per core across the
8 NeuronCores; the ~1MB weight set is replicated.

Per-core design:
- token tiles are (region n, batch-block of 128): partitions = batch, so
  attention (which needs all 5 regions of one batch on one partition) works
  directly with strided/broadcast APs on the free dim.
- 16:1 spatial mean pooling: DMA casts fp32->bf16 inline (SWDGE), then a
  binary add-tree on the vector engine (bf16 2x mode); the /16 is folded
  into w_in host-side.
- All matmuls on PE in bf16 (weights host-cast), accumulating fp32 in PSUM.
  Biases are added via rank-1 ones x bias matmuls, the residual via an
  identity matmul, so PSUM accumulates the whole pre-LN activation.
- LayerNorm stats ride along the PSUM-evacuation passes on the scalar engine
  (activation accum_out); rstd = rsqrt via DVE bit-trick + Newton so ScalarE
  stays on one activation-table set (no table-swap stalls).
- Work is emitted software-pipelined across batch blocks (A = proj+attention,
  B = LN tail) so each engine's in-order stream has no long dependency
  stalls: while one block's O-projection runs on PE/ACT, the next block's
  attention keeps the vector engine busy.
"""

import sys

sys.path.insert(0, "/opt/trn_rl_repo")

import numpy as np
import ml_dtypes

B, N, C, H, W = 4096, 5, 512, 4, 4
HW = H * W
HID, HEADS, LAYERS = 256, 4, 3
HD = HID // HEADS
NCORES = 8
P = 128

_BUILD_CACHE = {}


def _build(flags, BC):
    """Build the per-core Bass program. flags = (has_mask, has_b_in, has_b_qkv,
    has_b_o, has_ln_affine)."""
    import concourse.bass as bass
    import concourse.bacc as bacc
    import concourse.mybir as mybir
    import concourse.tile as tile
    from concourse.masks import make_identity

    has_mask, has_b_in, has_b_qkv, has_b_o, has_ln = flags
    f32 = mybir.dt.float32
    bf16 = mybir.dt.bfloat16
    i32 = mybir.dt.int32
    Alu = mybir.AluOpType
    Act = mybir.ActivationFunctionType

    NBLK = BC // P
    CH = C * HW  # 8192
    HNM = HEADS * N * N  # 100

    nc = bacc.Bacc("TRN2", target_bir_lowering=False, debug=False,
                   num_devices=NCORES)

    rf = nc.dram_tensor("rf", [BC, N, CH], f32, kind="ExternalInput")
    w_in_d = nc.dram_tensor("w_in_t", [4, P, HID], bf16, kind="ExternalInput")
    wqkv_d = nc.dram_tensor("wqkv_t", [LAYERS, 2, P, 3 * HID], bf16,
                            kind="ExternalInput")
    wo_d = nc.dram_tensor("wo_t", [LAYERS, 2, P, HID], bf16,
                          kind="ExternalInput")
    if has_b_in:
        b_in_d = nc.dram_tensor("b_in_t", [1, HID], bf16, kind="ExternalInput")
    if has_b_qkv:
        bqkv_d = nc.dram_tensor("bqkv_t", [LAYERS, 3 * HID], bf16,
                                kind="ExternalInput")
    if has_b_o:
        bo_d = nc.dram_tensor("bo_t", [LAYERS, HID], bf16, kind="ExternalInput")
    if has_ln:
        ln_g_d = nc.dram_tensor("ln_g_t", [LAYERS, HID], f32,
                                kind="ExternalInput")
        ln_b_d = nc.dram_tensor("ln_b_t", [LAYERS, HID], f32,
                                kind="ExternalInput")
    if has_mask:
        mask_d = nc.dram_tensor("mask_t", [N * N], f32, kind="ExternalInput")

    x_out = nc.dram_tensor("x_out", [BC, N, HID], f32, kind="ExternalOutput")
    attn_out = nc.dram_tensor("attn_out", [BC, HEADS * N * N], f32,
                              kind="ExternalOutput")

    def bcast_ap(base, off, dims):
        # custom AP over a tile: keep partition dim, replace free dims
        return bass.AP(tensor=base.tensor, offset=base.offset + off,
                       ap=[list(base.ap[0])] + [list(d) for d in dims])

    with tile.TileContext(nc) as tc:
        with (
            tc.tile_pool(name="consts", bufs=1) as consts,
            tc.tile_pool(name="xr", bufs=2) as xr_pool,
            tc.tile_pool(name="tree", bufs=1) as tree_pool,
            tc.tile_pool(name="pooled", bufs=2) as pooled_pool,
            tc.tile_pool(name="small", bufs=2) as small_pool,
            tc.tile_pool(name="xfer", bufs=6) as xfer_pool,
            tc.tile_pool(name="x", bufs=6) as x_pool,
            tc.tile_pool(name="qkv", bufs=3) as qkv_pool,
            # DVE-produced, DVE-consumed intermediates: bufs=1 (DVE is
            # serial anyway); cross-engine tiles get bufs>=2
            tc.tile_pool(name="attnd", bufs=1) as attnd_pool,
            tc.tile_pool(name="attnw", bufs=3) as attnw_pool,
            tc.tile_pool(name="stats", bufs=3) as stats_pool,
            tc.tile_pool(name="y", bufs=3) as y_pool,
            tc.tile_pool(name="psA", bufs=2, space="PSUM") as psA,
            tc.tile_pool(name="psT", bufs=1, space="PSUM") as psT,
            tc.tile_pool(name="psC", bufs=1, space="PSUM") as psC,
        ):
            # ---- constants ----
            ident32 = consts.tile([P, P], f32)
            make_identity(nc, ident32[:])
            identbf = consts.tile([P, P], bf16)
            make_identity(nc, identbf[:])
            magic_sb = consts.tile([P, N], i32)
            nc.vector.memset(magic_sb[:], 0x5F3759DF)

            w_in_sb = consts.tile([P, 4, HID], bf16)
            nc.sync.dma_start(out=w_in_sb[:],
                              in_=w_in_d[:].rearrange("k p o -> p k o"))
            wqkv_sb = consts.tile([P, LAYERS, 2, 3 * HID], bf16)
            nc.sync.dma_start(out=wqkv_sb[:],
                              in_=wqkv_d[:].rearrange("l k p o -> p l k o"))
            wo_sb = consts.tile([P, LAYERS, 2, HID], bf16)
            nc.sync.dma_start(out=wo_sb[:],
                              in_=wo_d[:].rearrange("l k p o -> p l k o"))

            if has_b_in or has_b_qkv or has_b_o:
                ones_row = consts.tile([1, P], bf16)
                nc.vector.memset(ones_row[:], 1.0)
            if has_b_in:
                b_in_sb = consts.tile([1, HID], bf16)
                nc.sync.dma_start(out=b_in_sb[:], in_=b_in_d[:])
            if has_b_qkv:
                bqkv_sb = consts.tile([1, LAYERS, 3 * HID], bf16)
                nc.sync.dma_start(out=bqkv_sb[:],
                                  in_=bqkv_d[:].rearrange("l o -> 1 l o"))
            if has_b_o:
                bo_sb = consts.tile([1, LAYERS, HID], bf16)
                nc.sync.dma_start(out=bo_sb[:],
                                  in_=bo_d[:].rearrange("l o -> 1 l o"))
            if has_ln:
                g_rep = consts.tile([P, LAYERS, HID], f32)
                nc.gpsimd.dma_start(
                    out=g_rep[:],
                    in_=bass.AP(tensor=ln_g_d, offset=0,
                                ap=[[0, P], [HID, LAYERS], [1, HID]]))
                b_rep = consts.tile([P, LAYERS, HID], f32)
                nc.gpsimd.dma_start(
                    out=b_rep[:],
                    in_=bass.AP(tensor=ln_b_d, offset=0,
                                ap=[[0, P], [HID, LAYERS], [1, HID]]))
            if has_mask:
                mask_rep = consts.tile([P, N * N], f32)
                nc.gpsimd.dma_start(
                    out=mask_rep[:],
                    in_=bass.AP(tensor=mask_d, offset=0,
                                ap=[[0, P], [1, N * N]]))

            xs = [None] * NBLK
            st = {}  # (l, blk) -> (y, sums, sumsq) handed from A to B

            def emit_p1(blk):
                """pooling + input projection -> xs[blk]"""
                b0 = blk * P
                x = x_pool.tile([P, N, HID], bf16, tag="x")
                xs[blk] = x
                for n in range(N):
                    xr = xr_pool.tile([P, C, HW], bf16)
                    nc.gpsimd.dma_start(
                        out=xr[:],
                        in_=rf[b0:b0 + P, n, :].rearrange("p (c h) -> p c h",
                                                          h=HW))
                    t1 = tree_pool.tile([P, C, 8], bf16, tag="t1")
                    nc.vector.tensor_add(t1[:], xr[:, :, 0:8], xr[:, :, 8:16])
                    t2 = tree_pool.tile([P, C, 4], bf16, tag="t2")
                    nc.vector.tensor_add(t2[:], t1[:, :, 0:4], t1[:, :, 4:8])
                    t3 = tree_pool.tile([P, C, 2], bf16, tag="t3")
                    nc.vector.tensor_add(t3[:], t2[:, :, 0:2], t2[:, :, 2:4])
                    pooled = pooled_pool.tile([P, C], f32, tag="pooled")
                    nc.vector.tensor_add(pooled[:], t3[:, :, 0], t3[:, :, 1])

                    pooledT_ps = psA.tile([P, 4, P], f32, tag="psA")
                    for j in range(4):
                        nc.tensor.transpose(pooledT_ps[:, j, :],
                                            pooled[:, j * P:(j + 1) * P],
                                            ident32[:])
                    pooledT = pooled_pool.tile([P, 4, P], bf16, tag="pooledT")
                    nc.scalar.copy(pooledT[:], pooledT_ps[:])

                    x_ps = psA.tile([P, HID], f32, tag="psA")
                    for j in range(4):
                        nc.tensor.matmul(x_ps[:], pooledT[:, j, :],
                                         w_in_sb[:, j, :], start=(j == 0),
                                         stop=(j == 3 and not has_b_in))
                    if has_b_in:
                        nc.tensor.matmul(x_ps[:], ones_row[:], b_in_sb[:],
                                         start=False, stop=True)
                    nc.scalar.copy(x[:, n, :], x_ps[:])

            def emit_A1(l, blk):
                """x transposes + QKV projection -> qkv"""
                x = xs[blk]
                qkv = qkv_pool.tile([P, N, 3, HID], bf16, tag="qkv")
                # transposes + evacuations first, then matmuls: keeps the
                # in-order ACT stream free of PE round-trip stalls
                xTs = []
                for n in range(N):
                    xT_ps = psT.tile([P, 2, P], bf16, tag="psT")
                    for j in range(2):
                        nc.tensor.transpose(xT_ps[:, j, :],
                                            x[:, n, j * P:(j + 1) * P],
                                            identbf[:])
                    xT = xfer_pool.tile([P, 2, P], bf16, tag="xT")
                    nc.scalar.copy(xT[:], xT_ps[:])
                    xTs.append(xT)
                for n in range(N):
                    xT = xTs[n]
                    qkv_ps = psA.tile([P, 3 * HID], f32, tag="psA")
                    for c0, c1 in ((0, 512), (512, 768)):
                        for j in range(2):
                            nc.tensor.matmul(qkv_ps[:, c0:c1], xT[:, j, :],
                                             wqkv_sb[:, l, j, c0:c1],
                                             start=(j == 0),
                                             stop=(j == 1 and not has_b_qkv))
                        if has_b_qkv:
                            nc.tensor.matmul(qkv_ps[:, c0:c1], ones_row[:],
                                             bqkv_sb[:, l, c0:c1],
                                             start=False, stop=True)
                    nc.scalar.copy(qkv[:, n, :, :], qkv_ps[:])
                st[("qkv", l, blk)] = qkv

            def emit_A2(l, blk):
                """attention (batch on partitions) -> ctx"""
                b0 = blk * P
                qkv = st.pop(("qkv", l, blk))
                # qkv free strides: n:768, slot:256, h:64, d:1
                qb = qkv[:]
                prod = attnd_pool.tile([P, HNM, HD], bf16, tag="bigprod")
                for h in range(HEADS):
                    # out block (n, m, d) at rows h*25..h*25+25
                    nc.vector.tensor_mul(
                        bcast_ap(prod[:], h * N * N * HD,
                                 [[N * HD, N], [HD, N], [1, HD]]),
                        bcast_ap(qb, h * HD,
                                 [[3 * HID, N], [0, N], [1, HD]]),
                        bcast_ap(qb, HID + h * HD,
                                 [[0, N], [3 * HID, N], [1, HD]]))
                s1 = attnd_pool.tile([P, HNM, 32], bf16, tag="s1")
                nc.vector.tensor_add(s1[:], prod[:, :, 0:32], prod[:, :, 32:64])
                s2 = attnd_pool.tile([P, HNM, 16], bf16, tag="s2")
                nc.vector.tensor_add(s2[:], s1[:, :, 0:16], s1[:, :, 16:32])
                s3 = attnd_pool.tile([P, HNM, 8], bf16, tag="s3")
                nc.vector.tensor_add(s3[:], s2[:, :, 0:8], s2[:, :, 8:16])
                s4 = attnd_pool.tile([P, HNM, 4], bf16, tag="s4")
                nc.vector.tensor_add(s4[:], s3[:, :, 0:4], s3[:, :, 4:8])
                s5 = attnd_pool.tile([P, HNM, 2], bf16, tag="s5")
                nc.vector.tensor_add(s5[:], s4[:, :, 0:2], s4[:, :, 2:4])
                scores = attnw_pool.tile([P, HEADS, N, N], bf16, tag="scores")
                nc.vector.tensor_add(
                    scores[:].rearrange("p h n m -> p (h n m)"),
                    s5[:, :, 0], s5[:, :, 1])
                if has_mask:
                    nc.vector.tensor_add(
                        scores[:], scores[:],
                        bcast_ap(mask_rep[:], 0, [[0, HEADS], [1, N * N]]))
                exps = attnw_pool.tile([P, HEADS, N, N], bf16, tag="exps")
                nc.scalar.activation(exps[:], scores[:], Act.Exp,
                                     scale=1.0 / np.sqrt(HD))
                Z = stats_pool.tile([P, HEADS * N], f32, tag="Z")
                nc.vector.tensor_reduce(
                    Z[:], exps[:].rearrange("p h n m -> p (h n) m"),
                    axis=mybir.AxisListType.X, op=Alu.add)
                rZ = stats_pool.tile([P, HEADS * N], f32, tag="rZ")
                nc.vector.reciprocal(rZ[:], Z[:])
                attnb = attnd_pool.tile([P, HEADS, N, N], bf16, tag="attnb")
                rz_ap = bcast_ap(rZ[:], 0, [[N, HEADS], [1, N], [0, N]])
                nc.vector.tensor_mul(attnb[:], exps[:], rz_ap)
                if l == LAYERS - 1:
                    attnf = attnw_pool.tile([P, HEADS, N, N], f32, tag="attnf")
                    nc.vector.tensor_mul(attnf[:], exps[:], rz_ap)
                    nc.sync.dma_start(
                        out=attn_out[b0:b0 + P, :],
                        in_=attnf[:].rearrange("p h n m -> p (h n m)"))

                # ctx[b, n, h, d] = sum_m attn[b,h,n,m] * V[b,m,h,d]
                # A2 = attn duplicated in d-pairs so products keep a step-1
                # even innermost dim (DVE 2x mode)
                A2 = attnd_pool.tile([P, HNM, 2], bf16, tag="A2")
                nc.vector.tensor_copy(
                    A2[:], bcast_ap(attnb[:], 0, [[1, HNM], [0, 2]]))
                # prod2 layout (n, h, m, d): strides n:1280, h:320, m:64, d:1
                prod2 = attnd_pool.tile([P, N * HEADS, N, HD], bf16,
                                        tag="bigprod")
                p2 = prod2[:]
                for h in range(HEADS):
                    for m in range(N):
                        nc.vector.tensor_mul(
                            bcast_ap(p2, h * N * HD + m * HD,
                                     [[HEADS * N * HD, N], [2, HD // 2],
                                      [1, 2]]),
                            bcast_ap(A2[:], (h * N * N + m) * 2,
                                     [[2 * N, N], [0, HD // 2], [1, 2]]),
                            bcast_ap(qb, 2 * HID + m * 3 * HID + h * HD,
                                     [[0, N], [2, HD // 2], [1, 2]]))
                # m-tree, d stays innermost -> all adds run 2x
                c1 = attnd_pool.tile([P, N * HEADS, 2, HD], bf16, tag="c1")
                nc.vector.tensor_add(
                    c1[:],
                    bcast_ap(p2, 0, [[N * HD, N * HEADS], [2 * HD, 2],
                                     [1, HD]]),
                    bcast_ap(p2, HD, [[N * HD, N * HEADS], [2 * HD, 2],
                                      [1, HD]]))
                c2 = attnd_pool.tile([P, N * HEADS, HD], bf16, tag="c2")
                nc.vector.tensor_add(c2[:], c1[:, :, 0, :], c1[:, :, 1, :])
                ctx = attnw_pool.tile([P, N, HID], bf16, tag="ctx")
                nc.vector.tensor_add(
                    ctx[:].rearrange("p n (h d) -> p (n h) d", d=HD),
                    c2[:],
                    bcast_ap(p2, 4 * HD, [[N * HD, N * HEADS], [1, HD]]))

                st[("ctx", l, blk)] = ctx

            def emit_A3(l, blk):
                """ctx transposes + O-projection + residual -> y"""
                x = xs[blk]
                ctx = st.pop(("ctx", l, blk))
                ctxTs = []
                for n in range(N):
                    ctxT_ps = psT.tile([P, 2, P], bf16, tag="psT")
                    for j in range(2):
                        nc.tensor.transpose(ctxT_ps[:, j, :],
                                            ctx[:, n, j * P:(j + 1) * P],
                                            identbf[:])
                    ctxT = xfer_pool.tile([P, 2, P], bf16, tag="ctxT")
                    nc.scalar.copy(ctxT[:], ctxT_ps[:])
                    ctxTs.append(ctxT)
                o_big = psC.tile([P, N, HID], f32, tag="psC")
                for n in range(N):
                    ctxT = ctxTs[n]
                    for j in range(2):
                        nc.tensor.matmul(o_big[:, n, :], ctxT[:, j, :],
                                         wo_sb[:, l, j, :],
                                         start=(j == 0), stop=False)
                    if has_b_o:
                        nc.tensor.matmul(o_big[:, n, :], ones_row[:],
                                         bo_sb[:, l, :], start=False,
                                         stop=False)
                    nc.tensor.matmul(o_big[:, n, :], identbf[:], x[:, n, :],
                                     start=False, stop=True)
                # one batched evacuation frees the PSUM slot early and lets
                # the LN tail read SBUF (faster DVE modes)
                y = y_pool.tile([P, N, HID], f32, tag="y")
                nc.scalar.copy(y[:], o_big[:])
                st[("y", l, blk)] = y

            def emit_B(l, blk):
                """LayerNorm tail -> xs[blk] (and final output DMA)"""
                b0 = blk * P
                y = st.pop(("y", l, blk))
                # per-token mean/variance on DVE
                mv = stats_pool.tile([P, N, 2], f32, tag="mv")
                for n in range(N):
                    bns = stats_pool.tile([P, 6], f32, tag="bns")
                    nc.vector.bn_stats(bns[:], y[:, n, :])
                    nc.vector.bn_aggr(mv[:, n, :], bns[:])
                var = stats_pool.tile([P, N], f32, tag="var")
                nc.vector.tensor_scalar_add(var[:], mv[:, :, 1], 1e-5)
                # rstd = 1/sqrt(var) on DVE: bit-trick + 3 Newton steps
                sh = stats_pool.tile([P, N], i32, tag="sh")
                nc.vector.tensor_scalar(sh[:], var[:].bitcast(i32), 1, None,
                                        op0=Alu.logical_shift_right)
                rstd = stats_pool.tile([P, N], f32, tag="rstd")
                nc.vector.tensor_sub(rstd[:].bitcast(i32), magic_sb[:], sh[:])
                for _ in range(3):
                    nt = stats_pool.tile([P, N], f32, tag="nt")
                    nc.vector.tensor_mul(nt[:], rstd[:], rstd[:])
                    nc.vector.tensor_mul(nt[:], nt[:], var[:])
                    nc.vector.tensor_scalar(nt[:], nt[:], -0.5, 1.5,
                                            op0=Alu.mult, op1=Alu.add)
                    nc.vector.tensor_mul(rstd[:], rstd[:], nt[:])

                final = l == LAYERS - 1
                if final:
                    # last layer's x goes straight to HBM: keep fp32
                    xn = x_pool.tile([P, N, HID], f32, tag="xf")
                else:
                    xn = x_pool.tile([P, N, HID], bf16, tag="x")
                # normalize on ScalarE: xn = y*rstd + (-mu*rstd)
                nmb = stats_pool.tile([P, N], f32, tag="nmb")
                nc.vector.scalar_tensor_tensor(nmb[:], mv[:, :, 0], -1.0,
                                               rstd[:], op0=Alu.mult,
                                               op1=Alu.mult)
                for n in range(N):
                    nc.scalar.activation(xn[:, n, :], y[:, n, :], Act.Identity,
                                         bias=nmb[:, n:n + 1],
                                         scale=rstd[:, n:n + 1])
                    if has_ln:
                        nc.vector.tensor_mul(xn[:, n, :], xn[:, n, :],
                                             g_rep[:, l, :])
                        nc.vector.tensor_add(xn[:, n, :], xn[:, n, :],
                                             b_rep[:, l, :])
                xs[blk] = xn
                if l == LAYERS - 1:
                    nc.sync.dma_start(out=x_out[b0:b0 + P, :, :], in_=xn[:])

            # ---- software-pipelined emission ----
            # Slot k handles A-stage (l,blk) = S[k]. Emitting next slot's QKV
            # (A1) before this slot's O-projection (A3), and LayerNorm (B)
            # two slots late, keeps every engine's in-order stream stall-free:
            # PE/ACT always have independent work while DVE runs attention.
            if NBLK == 1:
                emit_p1(0)
                for l in range(LAYERS):
                    emit_A1(l, 0)
                    emit_A2(l, 0)
                    emit_A3(l, 0)
                    emit_B(l, 0)
            else:
                S = [(l, blk) for l in range(LAYERS) for blk in range(NBLK)]
                emit_p1(0)
                emit_p1(1)
                for k in range(len(S)):
                    if k >= 2:
                        emit_B(*S[k - 2])
                    emit_A1(*S[k])
                    if k + 2 < NBLK:
                        emit_p1(k + 2)
                    if k >= 1:
                        emit_A3(*S[k - 1])
                    emit_A2(*S[k])
                emit_A3(*S[-1])
                emit_B(*S[-2])
                emit_B(*S[-1])

    nc.compile()
    return nc


def _get_program(flags, BC):
    key = (flags, BC)
    if key not in _BUILD_CACHE:
        _BUILD_CACHE[key] = _build(flags, BC)
    return _BUILD_CACHE[key]


def kernel(region_features, adjacency, w_in, b_in, wq, bq, wk, bk, wv, bv,
           wo, bo, ln_g, ln_b, _trace=False, _bc=None):
    from concourse.bass_utils import run_bass_kernel_spmd

    region_features = np.asarray(region_features)
    adjacency = np.asarray(adjacency, dtype=np.float32)
    w_in = np.asarray(w_in, dtype=np.float32)
    b_in = np.asarray(b_in, dtype=np.float32)
    wq, wk, wv, wo = (np.asarray(a, dtype=np.float32) for a in (wq, wk, wv, wo))
    bq, bk, bv, bo = (np.asarray(a, dtype=np.float32) for a in (bq, bk, bv, bo))
    ln_g = np.asarray(ln_g, dtype=np.float32)
    ln_b = np.asarray(ln_b, dtype=np.float32)

    Btot = region_features.shape[0]
    BC = _bc if _bc is not None else Btot // NCORES
    assert Btot == BC * NCORES

    has_mask = bool((adjacency == 0).any())
    has_b_in = bool(np.any(b_in != 0))
    has_b_qkv = bool(np.any(bq != 0) or np.any(bk != 0) or np.any(bv != 0))
    has_b_o = bool(np.any(bo != 0))
    has_ln = bool(np.any(ln_g != 1) or np.any(ln_b != 0))
    flags = (has_mask, has_b_in, has_b_qkv, has_b_o, has_ln)

    nc = _get_program(flags, BC)

    bf = ml_dtypes.bfloat16
    # host-side weight prep: fold /16 pooling into w_in, chunk contraction dim
    w_in_t = np.ascontiguousarray((w_in / HW).reshape(4, P, HID)).astype(bf)
    wqkv = np.concatenate([wq, wk, wv], axis=2)  # [L, 256, 768]
    wqkv_t = np.ascontiguousarray(wqkv.reshape(LAYERS, 2, P, 3 * HID)).astype(bf)
    wo_t = np.ascontiguousarray(wo.reshape(LAYERS, 2, P, HID)).astype(bf)

    base_map = {
        "w_in_t": w_in_t,
        "wqkv_t": wqkv_t,
        "wo_t": wo_t,
    }
    if has_b_in:
        base_map["b_in_t"] = b_in.reshape(1, HID).astype(bf)
    if has_b_qkv:
        base_map["bqkv_t"] = np.concatenate([bq, bk, bv], axis=1).astype(bf)
    if has_b_o:
        base_map["bo_t"] = bo.astype(bf)
    if has_ln:
        base_map["ln_g_t"] = ln_g
        base_map["ln_b_t"] = ln_b
    if has_mask:
        base_map["mask_t"] = np.where(adjacency == 0, -1e9,
                                      0.0).astype(np.float32).reshape(N * N)

    rf_flat = np.ascontiguousarray(
        region_features.reshape(Btot, N, C * HW)).astype(np.float32)
    in_maps = []
    for c in range(NCORES):
        m = dict(base_map)
        m["rf"] = rf_flat[c * BC:(c + 1) * BC]
        in_maps.append(m)

    res = run_bass_kernel_spmd(nc, in_maps, core_ids=list(range(NCORES)),
                               trace=_trace)
    kernel.last_results = res

    x_full = np.concatenate([r["x_out"] for r in res.results], axis=0)
    attn_full = np.concatenate(
        [r["attn_out"].reshape(BC, HEADS, N, N) for r in res.results], axis=0)
    return x_full, attn_full


# revision 33
# speedup vs baseline: 1.0210x; 1.0123x over previous
"""Trainium2 Bass kernel for CrossRegionRelationalReasoning (gnn_message_passing).

Computation: mean-pool [B,N,C,4,4] -> [B,N,C], project to HID, then 3 layers of
tiny self-attention (N=5 regions, 4 heads) with residual + LayerNorm.
Returns (x, attn) like the reference.

Sharding: pure data parallel. B=4096 split as 512 batches per core across the
8 NeuronCores; the ~1MB weight set is replicated.

Per-core design:
- token tiles are (region n, batch-block of 128): partitions = batch, so
  attention (which needs all 5 regions of one batch on one partition) works
  directly with strided/broadcast APs on the free dim.
- 16:1 spatial mean pooling: DMA casts fp32->bf16 inline (SWDGE), then a
  binary add-tree on the vector engine (bf16 2x mode); the /16 is folded
  into w_in host-side.
- All matmuls on PE in bf16 (weights host-cast), accumulating fp32 in PSUM.
  Biases are added via rank-1 ones x bias matmuls, the residual via an
  identity matmul, so PSUM accumulates the whole pre-LN activation.
- LayerNorm: per-token stats via DVE bn_stats/bn_aggr, rstd via DVE
  bit-trick rsqrt + Newton (keeps ScalarE on one activation-table set, no
  table-swap stalls), normalize via ScalarE Identity-activation with
  per-partition scale/bias.
- Attention DVE ops are shaped so every tensor_tensor runs in the bf16 2x
  perf mode (step-1 even innermost dims; attn weights pair-duplicated).
- Work is emitted software-pipelined across batch blocks in slots
  (A1 = QKV, A2 = attention, A3 = O-proj, B = LN two slots later) so each
  engine's strictly in-order stream never waits on a fresh cross-engine
  dependency: PE always has the next block's QKV before the previous
  block's O-projection, and the vector engine runs at ~85% occupancy.
"""

import sys

sys.path.insert(0, "/opt/trn_rl_repo")

import numpy as np
import ml_dtypes

B, N, C, H, W = 4096, 5, 512, 4, 4
HW = H * W
HID, HEADS, LAYERS = 256, 4, 3
HD = HID // HEADS
NCORES = 8
P = 128

_BUILD_CACHE = {}


def _build(flags, BC):
    """Build the per-core Bass program. flags = (has_mask, has_b_in, has_b_qkv,
    has_b_o, has_ln_affine)."""
    import concourse.bass as bass
    import concourse.bacc as bacc
    import concourse.mybir as mybir
    import concourse.tile as tile
    from concourse.masks import make_identity

    has_mask, has_b_in, has_b_qkv, has_b_o, has_ln = flags
    f32 = mybir.dt.float32
    bf16 = mybir.dt.bfloat16
    i32 = mybir.dt.int32
    Alu = mybir.AluOpType
    Act = mybir.ActivationFunctionType

    NBLK = BC // P
    CH = C * HW  # 8192
    HNM = HEADS * N * N  # 100

    nc = bacc.Bacc("TRN2", target_bir_lowering=False, debug=False,
                   num_devices=NCORES)

    rf = nc.dram_tensor("rf", [BC, N, CH], f32, kind="ExternalInput")
    w_in_d = nc.dram_tensor("w_in_t", [4, P, HID], bf16, kind="ExternalInput")
    wqkv_d = nc.dram_tensor("wqkv_t", [LAYERS, 2, P, 3 * HID], bf16,
                            kind="ExternalInput")
    wo_d = nc.dram_tensor("wo_t", [LAYERS, 2, P, HID], bf16,
                          kind="ExternalInput")
    if has_b_in:
        b_in_d = nc.dram_tensor("b_in_t", [1, HID], bf16, kind="ExternalInput")
    if has_b_qkv:
        bqkv_d = nc.dram_tensor("bqkv_t", [LAYERS, 3 * HID], bf16,
                                kind="ExternalInput")
    if has_b_o:
        bo_d = nc.dram_tensor("bo_t", [LAYERS, HID], bf16, kind="ExternalInput")
    if has_ln:
        ln_g_d = nc.dram_tensor("ln_g_t", [LAYERS, HID], f32,
                                kind="ExternalInput")
        ln_b_d = nc.dram_tensor("ln_b_t", [LAYERS, HID], f32,
                                kind="ExternalInput")
    if has_mask:
        mask_d = nc.dram_tensor("mask_t", [N * N], f32, kind="ExternalInput")

    x_out = nc.dram_tensor("x_out", [BC, N, HID], f32, kind="ExternalOutput")
    attn_out = nc.dram_tensor("attn_out", [BC, HEADS * N * N], f32,
                              kind="ExternalOutput")

    def bcast_ap(base, off, dims):
        # custom AP over a tile: keep partition dim, replace free dims
        return bass.AP(tensor=base.tensor, offset=base.offset + off,
                       ap=[list(base.ap[0])] + [list(d) for d in dims])

    with tile.TileContext(nc) as tc:
        with (
            tc.tile_pool(name="consts", bufs=1) as consts,
            tc.tile_pool(name="xr", bufs=2) as xr_pool,
            tc.tile_pool(name="tree", bufs=1) as tree_pool,
            tc.tile_pool(name="pooled", bufs=2) as pooled_pool,
            tc.tile_pool(name="small", bufs=2) as small_pool,
            tc.tile_pool(name="xfer", bufs=6) as xfer_pool,
            tc.tile_pool(name="x", bufs=6) as x_pool,
            tc.tile_pool(name="qkv", bufs=3) as qkv_pool,
            # DVE-produced, DVE-consumed intermediates: bufs=1 (DVE is
            # serial anyway); cross-engine tiles get bufs>=2
            tc.tile_pool(name="attnd", bufs=1) as attnd_pool,
            tc.tile_pool(name="attnw", bufs=3) as attnw_pool,
            tc.tile_pool(name="stats", bufs=3) as stats_pool,
            tc.tile_pool(name="y", bufs=3) as y_pool,
            tc.tile_pool(name="psA", bufs=2, space="PSUM") as psA,
            tc.tile_pool(name="psT", bufs=1, space="PSUM") as psT,
            tc.tile_pool(name="psC", bufs=1, space="PSUM") as psC,
        ):
            # ---- constants ----
            ident32 = consts.tile([P, P], f32)
            make_identity(nc, ident32[:])
            identbf = consts.tile([P, P], bf16)
            make_identity(nc, identbf[:])
            magic_sb = consts.tile([P, N], i32)
            nc.vector.memset(magic_sb[:], 0x5F3759DF)

            w_in_sb = consts.tile([P, 4, HID], bf16)
            nc.sync.dma_start(out=w_in_sb[:],
                              in_=w_in_d[:].rearrange("k p o -> p k o"))
            wqkv_sb = consts.tile([P, LAYERS, 2, 3 * HID], bf16)
            nc.sync.dma_start(out=wqkv_sb[:],
                              in_=wqkv_d[:].rearrange("l k p o -> p l k o"))
            wo_sb = consts.tile([P, LAYERS, 2, HID], bf16)
            nc.sync.dma_start(out=wo_sb[:],
                              in_=wo_d[:].rearrange("l k p o -> p l k o"))

            if has_b_in or has_b_qkv or has_b_o:
                ones_row = consts.tile([1, P], bf16)
                nc.vector.memset(ones_row[:], 1.0)
            if has_b_in:
                b_in_sb = consts.tile([1, HID], bf16)
                nc.sync.dma_start(out=b_in_sb[:], in_=b_in_d[:])
            if has_b_qkv:
                bqkv_sb = consts.tile([1, LAYERS, 3 * HID], bf16)
                nc.sync.dma_start(out=bqkv_sb[:],
                                  in_=bqkv_d[:].rearrange("l o -> 1 l o"))
            if has_b_o:
                bo_sb = consts.tile([1, LAYERS, HID], bf16)
                nc.sync.dma_start(out=bo_sb[:],
                                  in_=bo_d[:].rearrange("l o -> 1 l o"))
            if has_ln:
                g_rep = consts.tile([P, LAYERS, HID], f32)
                nc.gpsimd.dma_start(
                    out=g_rep[:],
                    in_=bass.AP(tensor=ln_g_d, offset=0,
                                ap=[[0, P], [HID, LAYERS], [1, HID]]))
                b_rep = consts.tile([P, LAYERS, HID], f32)
                nc.gpsimd.dma_start(
                    out=b_rep[:],
                    in_=bass.AP(tensor=ln_b_d, offset=0,
                                ap=[[0, P], [HID, LAYERS], [1, HID]]))
            if has_mask:
                mask_rep = consts.tile([P, N * N], f32)
                nc.gpsimd.dma_start(
                    out=mask_rep[:],
                    in_=bass.AP(tensor=mask_d, offset=0,
                                ap=[[0, P], [1, N * N]]))

            xs = [None] * NBLK
            st = {}  # (l, blk) -> (y, sums, sumsq) handed from A to B

            def emit_p1(blk):
                """pooling + input projection -> xs[blk]"""
                b0 = blk * P
                x = x_pool.tile([P, N, HID], bf16, tag="x")
                xs[blk] = x
                for n in range(N):
                    xr = xr_pool.tile([P, C, HW], bf16)
                    nc.gpsimd.dma_start(
                        out=xr[:],
                        in_=rf[b0:b0 + P, n, :].rearrange("p (c h) -> p c h",
                                                          h=HW))
                    t1 = tree_pool.tile([P, C, 8], bf16, tag="t1")
                    nc.vector.tensor_add(t1[:], xr[:, :, 0:8], xr[:, :, 8:16])
                    t2 = tree_pool.tile([P, C, 4], bf16, tag="t2")
                    nc.vector.tensor_add(t2[:], t1[:, :, 0:4], t1[:, :, 4:8])
                    t3 = tree_pool.tile([P, C, 2], bf16, tag="t3")
                    nc.vector.tensor_add(t3[:], t2[:, :, 0:2], t2[:, :, 2:4])
                    pooled = pooled_pool.tile([P, C], f32, tag="pooled")
                    nc.vector.tensor_add(pooled[:], t3[:, :, 0], t3[:, :, 1])

                    pooledT_ps = psA.tile([P, 4, P], f32, tag="psA")
                    for j in range(4):
                        nc.tensor.transpose(pooledT_ps[:, j, :],
                                            pooled[:, j * P:(j + 1) * P],
                                            ident32[:])
                    pooledT = pooled_pool.tile([P, 4, P], bf16, tag="pooledT")
                    nc.scalar.copy(pooledT[:], pooledT_ps[:])

                    x_ps = psA.tile([P, HID], f32, tag="psA")
                    for j in range(4):
                        nc.tensor.matmul(x_ps[:], pooledT[:, j, :],
                                         w_in_sb[:, j, :], start=(j == 0),
                                         stop=(j == 3 and not has_b_in))
                    if has_b_in:
                        nc.tensor.matmul(x_ps[:], ones_row[:], b_in_sb[:],
                                         start=False, stop=True)
                    nc.scalar.copy(x[:, n, :], x_ps[:])

            def emit_A1(l, blk):
                """x transposes + QKV projection -> qkv"""
                x = xs[blk]
                qkv = qkv_pool.tile([P, N, 3, HID], bf16, tag="qkv")
                # transposes + evacuations first, then matmuls: keeps the
                # in-order ACT stream free of PE round-trip stalls
                xTs = []
                for n in range(N):
                    xT_ps = psT.tile([P, 2, P], bf16, tag="psT")
                    for j in range(2):
                        nc.tensor.transpose(xT_ps[:, j, :],
                                            x[:, n, j * P:(j + 1) * P],
                                            identbf[:])
                    xT = xfer_pool.tile([P, 2, P], bf16, tag="xT")
                    nc.scalar.copy(xT[:], xT_ps[:])
                    xTs.append(xT)
                for n in range(N):
                    xT = xTs[n]
                    qkv_ps = psA.tile([P, 3 * HID], f32, tag="psA")
                    for c0, c1 in ((0, 512), (512, 768)):
                        for j in range(2):
                            nc.tensor.matmul(qkv_ps[:, c0:c1], xT[:, j, :],
                                             wqkv_sb[:, l, j, c0:c1],
                                             start=(j == 0),
                                             stop=(j == 1 and not has_b_qkv))
                        if has_b_qkv:
                            nc.tensor.matmul(qkv_ps[:, c0:c1], ones_row[:],
                                             bqkv_sb[:, l, c0:c1],
                                             start=False, stop=True)
                    nc.scalar.copy(qkv[:, n, :, :], qkv_ps[:])
                st[("qkv", l, blk)] = qkv

            def emit_A2(l, blk):
                """attention (batch on partitions) -> ctx"""
                b0 = blk * P
                qkv = st.pop(("qkv", l, blk))
                # qkv free strides: n:768, slot:256, h:64, d:1
                qb = qkv[:]
                prod = attnd_pool.tile([P, HNM, HD], bf16, tag="bigprod")
                for h in range(HEADS):
                    # out block (n, m, d) at rows h*25..h*25+25
                    nc.vector.tensor_mul(
                        bcast_ap(prod[:], h * N * N * HD,
                                 [[N * HD, N], [HD, N], [1, HD]]),
                        bcast_ap(qb, h * HD,
                                 [[3 * HID, N], [0, N], [1, HD]]),
                        bcast_ap(qb, HID + h * HD,
                                 [[0, N], [3 * HID, N], [1, HD]]))
                s1 = attnd_pool.tile([P, HNM, 32], bf16, tag="s1")
                nc.vector.tensor_add(s1[:], prod[:, :, 0:32], prod[:, :, 32:64])
                s2 = attnd_pool.tile([P, HNM, 16], bf16, tag="s2")
                nc.vector.tensor_add(s2[:], s1[:, :, 0:16], s1[:, :, 16:32])
                s3 = attnd_pool.tile([P, HNM, 8], bf16, tag="s3")
                nc.vector.tensor_add(s3[:], s2[:, :, 0:8], s2[:, :, 8:16])
                s4 = attnd_pool.tile([P, HNM, 4], bf16, tag="s4")
                nc.vector.tensor_add(s4[:], s3[:, :, 0:4], s3[:, :, 4:8])
                s5 = attnd_pool.tile([P, HNM, 2], bf16, tag="s5")
                nc.vector.tensor_add(s5[:], s4[:, :, 0:2], s4[:, :, 2:4])
                scores = attnw_pool.tile([P, HEADS, N, N], bf16, tag="scores")
                nc.vector.tensor_add(
                    scores[:].rearrange("p h n m -> p (h n m)"),
                    s5[:, :, 0], s5[:, :, 1])
                if has_mask:
                    nc.vector.tensor_add(
                        scores[:], scores[:],
                        bcast_ap(mask_rep[:], 0, [[0, HEADS], [1, N * N]]))
                exps = attnw_pool.tile([P, HEADS, N, N], bf16, tag="exps")
                nc.scalar.activation(exps[:], scores[:], Act.Exp,
                                     scale=1.0 / np.sqrt(HD))
                Z = stats_pool.tile([P, HEADS * N], f32, tag="Z")
                nc.vector.tensor_reduce(
                    Z[:], exps[:].rearrange("p h n m -> p (h n) m"),
                    axis=mybir.AxisListType.X, op=Alu.add)
                rZ = stats_pool.tile([P, HEADS * N], f32, tag="rZ")
                nc.vector.reciprocal(rZ[:], Z[:])
                attnb = attnd_pool.tile([P, HEADS, N, N], bf16, tag="attnb")
                rz_ap = bcast_ap(rZ[:], 0, [[N, HEADS], [1, N], [0, N]])
                nc.vector.tensor_mul(attnb[:], exps[:], rz_ap)
                if l == LAYERS - 1:
                    attnf = attnw_pool.tile([P, HEADS, N, N], f32, tag="attnf")
                    nc.vector.tensor_mul(attnf[:], exps[:], rz_ap)
                    nc.sync.dma_start(
                        out=attn_out[b0:b0 + P, :],
                        in_=attnf[:].rearrange("p h n m -> p (h n m)"))

                # ctx[b, n, h, d] = sum_m attn[b,h,n,m] * V[b,m,h,d]
                # A2 = attn duplicated in d-pairs so products keep a step-1
                # even innermost dim (DVE 2x mode)
                A2 = attnd_pool.tile([P, HNM, 2], bf16, tag="A2")
                nc.vector.tensor_copy(
                    A2[:], bcast_ap(attnb[:], 0, [[1, HNM], [0, 2]]))
                # prod2 layout (n, h, m, d): strides n:1280, h:320, m:64, d:1
                prod2 = attnd_pool.tile([P, N * HEADS, N, HD], bf16,
                                        tag="bigprod")
                p2 = prod2[:]
                for h in range(HEADS):
                    for m in range(N):
                        nc.vector.tensor_mul(
                            bcast_ap(p2, h * N * HD + m * HD,
                                     [[HEADS * N * HD, N], [2, HD // 2],
                                      [1, 2]]),
                            bcast_ap(A2[:], (h * N * N + m) * 2,
                                     [[2 * N, N], [0, HD // 2], [1, 2]]),
                            bcast_ap(qb, 2 * HID + m * 3 * HID + h * HD,
                                     [[0, N], [2, HD // 2], [1, 2]]))
                # m-tree, d stays innermost -> all adds run 2x
                c1 = attnd_pool.tile([P, N * HEADS, 2, HD], bf16, tag="c1")
                nc.vector.tensor_add(
                    c1[:],
                    bcast_ap(p2, 0, [[N * HD, N * HEADS], [2 * HD, 2],
                                     [1, HD]]),
                    bcast_ap(p2, HD, [[N * HD, N * HEADS], [2 * HD, 2],
                                      [1, HD]]))
                c2 = attnd_pool.tile([P, N * HEADS, HD], bf16, tag="c2")
                nc.vector.tensor_add(c2[:], c1[:, :, 0, :], c1[:, :, 1, :])
                ctx = attnw_pool.tile([P, N, HID], bf16, tag="ctx")
                nc.vector.tensor_add(
                    ctx[:].rearrange("p n (h d) -> p (n h) d", d=HD),
                    c2[:],
                    bcast_ap(p2, 4 * HD, [[N * HD, N * HEADS], [1, HD]]))

                st[("ctx", l, blk)] = ctx

            def emit_A3(l, blk):
                """ctx transposes + O-projection + residual -> y"""
                x = xs[blk]
                ctx = st.pop(("ctx", l, blk))
                ctxTs = []
                for n in range(N):
                    ctxT_ps = psT.tile([P, 2, P], bf16, tag="psT")
                    for j in range(2):
                        nc.tensor.transpose(ctxT_ps[:, j, :],
                                            ctx[:, n, j * P:(j + 1) * P],
                                            identbf[:])
                    ctxT = xfer_pool.tile([P, 2, P], bf16, tag="ctxT")
                    nc.scalar.copy(ctxT[:], ctxT_ps[:])
                    ctxTs.append(ctxT)
                o_big = psC.tile([P, N, HID], f32, tag="psC")
                for n in range(N):
                    ctxT = ctxTs[n]
                    for j in range(2):
                        nc.tensor.matmul(o_big[:, n, :], ctxT[:, j, :],
                                         wo_sb[:, l, j, :],
                                         start=(j == 0), stop=False)
                    if has_b_o:
                        nc.tensor.matmul(o_big[:, n, :], ones_row[:],
                                         bo_sb[:, l, :], start=False,
                                         stop=False)
                    nc.tensor.matmul(o_big[:, n, :], identbf[:], x[:, n, :],
                                     start=False, stop=True)
                # one batched evacuation frees the PSUM slot early and lets
                # the LN tail read SBUF (faster DVE modes)
                y = y_pool.tile([P, N, HID], f32, tag="y")
                nc.scalar.copy(y[:], o_big[:])
                st[("y", l, blk)] = y

            def emit_B(l, blk):
                """LayerNorm tail -> xs[blk] (and final output DMA)"""
                b0 = blk * P
                y = st.pop(("y", l, blk))
                # per-token mean/variance on DVE
                mv = stats_pool.tile([P, N, 2], f32, tag="mv")
                for n in range(N):
                    bns = stats_pool.tile([P, 6], f32, tag="bns")
                    nc.vector.bn_stats(bns[:], y[:, n, :])
                    nc.vector.bn_aggr(mv[:, n, :], bns[:])
                var = stats_pool.tile([P, N], f32, tag="var")
                nc.vector.tensor_scalar_add(var[:], mv[:, :, 1], 1e-5)
                # rstd = 1/sqrt(var) on DVE: bit-trick + 3 Newton steps
                sh = stats_pool.tile([P, N], i32, tag="sh")
                nc.vector.tensor_scalar(sh[:], var[:].bitcast(i32), 1, None,
                                        op0=Alu.logical_shift_right)
                rstd = stats_pool.tile([P, N], f32, tag="rstd")
                nc.vector.tensor_sub(rstd[:].bitcast(i32), magic_sb[:], sh[:])
                for _ in range(2):
                    nt = stats_pool.tile([P, N], f32, tag="nt")
                    nc.vector.tensor_mul(nt[:], rstd[:], rstd[:])
                    nc.vector.tensor_mul(nt[:], nt[:], var[:])
                    nc.vector.tensor_scalar(nt[:], nt[:], -0.5, 1.5,
                                            op0=Alu.mult, op1=Alu.add)
                    nc.vector.tensor_mul(rstd[:], rstd[:], nt[:])

                final = l == LAYERS - 1
                if final:
                    # last layer's x goes straight to HBM: keep fp32
                    xn = x_pool.tile([P, N, HID], f32, tag="xf")
                else:
                    xn = x_pool.tile([P, N, HID], bf16, tag="x")
                # normalize on ScalarE: xn = y*rstd + (-mu*rstd)
                nmb = stats_pool.tile([P, N], f32, tag="nmb")
                nc.vector.scalar_tensor_tensor(nmb[:], mv[:, :, 0], -1.0,
                                               rstd[:], op0=Alu.mult,
                                               op1=Alu.mult)
                for n in range(N):
                    nc.scalar.activation(xn[:, n, :], y[:, n, :], Act.Identity,
                                         bias=nmb[:, n:n + 1],
                                         scale=rstd[:, n:n + 1])
                    if has_ln:
                        nc.vector.tensor_mul(xn[:, n, :], xn[:, n, :],
                                             g_rep[:, l, :])
                        nc.vector.tensor_add(xn[:, n, :], xn[:, n, :],
                                             b_rep[:, l, :])
                xs[blk] = xn
                if l == LAYERS - 1:
                    nc.sync.dma_start(out=x_out[b0:b0 + P, :, :], in_=xn[:])

            # ---- software-pipelined emission ----
            # Slot k handles A-stage (l,blk) = S[k]. Emitting next slot's QKV
            # (A1) before this slot's O-projection (A3), and LayerNorm (B)
            # two slots late, keeps every engine's in-order stream stall-free:
            # PE/ACT always have independent work while DVE runs attention.
            if NBLK == 1:
                emit_p1(0)
                for l in range(LAYERS):
                    emit_A1(l, 0)
                    emit_A2(l, 0)
                    emit_A3(l, 0)
                    emit_B(l, 0)
            else:
                S = [(l, blk) for l in range(LAYERS) for blk in range(NBLK)]
                emit_p1(0)
                emit_p1(1)
                for k in range(len(S)):
                    if k >= 2:
                        emit_B(*S[k - 2])
                    emit_A1(*S[k])
                    if k + 2 < NBLK:
                        emit_p1(k + 2)
                    if k >= 1:
                        emit_A3(*S[k - 1])
                    emit_A2(*S[k])
                emit_A3(*S[-1])
                emit_B(*S[-2])
                emit_B(*S[-1])

    nc.compile()
    return nc


def _get_program(flags, BC):
    key = (flags, BC)
    if key not in _BUILD_CACHE:
        _BUILD_CACHE[key] = _build(flags, BC)
    return _BUILD_CACHE[key]


def kernel(region_features, adjacency, w_in, b_in, wq, bq, wk, bk, wv, bv,
           wo, bo, ln_g, ln_b, _trace=False, _bc=None):
    from concourse.bass_utils import run_bass_kernel_spmd

    region_features = np.asarray(region_features)
    adjacency = np.asarray(adjacency, dtype=np.float32)
    w_in = np.asarray(w_in, dtype=np.float32)
    b_in = np.asarray(b_in, dtype=np.float32)
    wq, wk, wv, wo = (np.asarray(a, dtype=np.float32) for a in (wq, wk, wv, wo))
    bq, bk, bv, bo = (np.asarray(a, dtype=np.float32) for a in (bq, bk, bv, bo))
    ln_g = np.asarray(ln_g, dtype=np.float32)
    ln_b = np.asarray(ln_b, dtype=np.float32)

    Btot = region_features.shape[0]
    BC = _bc if _bc is not None else Btot // NCORES
    assert Btot == BC * NCORES

    has_mask = bool((adjacency == 0).any())
    has_b_in = bool(np.any(b_in != 0))
    has_b_qkv = bool(np.any(bq != 0) or np.any(bk != 0) or np.any(bv != 0))
    has_b_o = bool(np.any(bo != 0))
    has_ln = bool(np.any(ln_g != 1) or np.any(ln_b != 0))
    flags = (has_mask, has_b_in, has_b_qkv, has_b_o, has_ln)

    nc = _get_program(flags, BC)

    bf = ml_dtypes.bfloat16
    # host-side weight prep: fold /16 pooling into w_in, chunk contraction dim
    w_in_t = np.ascontiguousarray((w_in / HW).reshape(4, P, HID)).astype(bf)
    wqkv = np.concatenate([wq, wk, wv], axis=2)  # [L, 256, 768]
    wqkv_t = np.ascontiguousarray(wqkv.reshape(LAYERS, 2, P, 3 * HID)).astype(bf)
    wo_t = np.ascontiguousarray(wo.reshape(LAYERS, 2, P, HID)).astype(bf)

    base_map = {
        "w_in_t": w_in_t,
        "wqkv_t": wqkv_t,
        "wo_t": wo_t,
    }
    if has_b_in:
        base_map["b_in_t"] = b_in.reshape(1, HID).astype(bf)
    if has_b_qkv:
        base_map["bqkv_t"] = np.concatenate([bq, bk, bv], axis=1).astype(bf)
    if has_b_o:
        base_map["bo_t"] = bo.astype(bf)
    if has_ln:
        base_map["ln_g_t"] = ln_g
        base_map["ln_b_t"] = ln_b
    if has_mask:
        base_map["mask_t"] = np.where(adjacency == 0, -1e9,
                                      0.0).astype(np.float32).reshape(N * N)

    rf_flat = np.ascontiguousarray(
        region_features.reshape(Btot, N, C * HW)).astype(np.float32)
    in_maps = []
    for c in range(NCORES):
        m = dict(base_map)
        m["rf"] = rf_flat[c * BC:(c + 1) * BC]
        in_maps.append(m)

    res = run_bass_kernel_spmd(nc, in_maps, core_ids=list(range(NCORES)),
                               trace=_trace)
    kernel.last_results = res

    x_full = np.concatenate([r["x_out"] for r in res.results], axis=0)
    attn_full = np.concatenate(
        [r["attn_out"].reshape(BC, HEADS, N, N) for r in res.results], axis=0)
    return x_full, attn_full


# revision 34
# speedup vs baseline: 1.0210x; 1.0000x over previous
"""Trainium2 Bass kernel for CrossRegionRelationalReasoning (gnn_message_passing).

Computation: mean-pool [B,N,C,4,4] -> [B,N,C], project to HID, then 3 layers of
tiny self-attention (N=5 regions, 4 heads) with residual + LayerNorm.
Returns (x, attn) like the reference.

Sharding: pure data parallel. B=4096 split as 512 batches per core across the
8 NeuronCores; the ~1MB weight set is replicated.

Per-core design:
- token tiles are (region n, batch-block of 128): partitions = batch, so
  attention (which needs all 5 regions of one batch on one partition) works
  directly with strided/broadcast APs on the free dim.
- 16:1 spatial mean pooling: DMA casts fp32->bf16 inline (SWDGE), then a
  binary add-tree on the vector engine (bf16 2x mode); the /16 is folded
  into w_in host-side.
- All matmuls on PE in bf16 (weights host-cast), accumulating fp32 in PSUM.
  Biases are added via rank-1 ones x bias matmuls, the residual via an
  identity matmul, so PSUM accumulates the whole pre-LN activation.
- LayerNorm: per-token stats via DVE bn_stats/bn_aggr, rstd via DVE
  bit-trick rsqrt + Newton (keeps ScalarE on one activation-table set, no
  table-swap stalls), normalize via ScalarE Identity-activation with
  per-partition scale/bias.
- Attention DVE ops are shaped so every tensor_tensor runs in the bf16 2x
  perf mode (step-1 even innermost dims; attn weights pair-duplicated).
- Work is emitted software-pipelined across batch blocks in slots
  (A1 = QKV, A2 = attention, A3 = O-proj, B = LN two slots later) so each
  engine's strictly in-order stream never waits on a fresh cross-engine
  dependency: PE always has the next block's QKV before the previous
  block's O-projection, and the vector engine runs at ~85% occupancy.
"""

import sys

sys.path.insert(0, "/opt/trn_rl_repo")

import numpy as np
import ml_dtypes

B, N, C, H, W = 4096, 5, 512, 4, 4
HW = H * W
HID, HEADS, LAYERS = 256, 4, 3
HD = HID // HEADS
NCORES = 8
P = 128

_BUILD_CACHE = {}


def _build(flags, BC):
    """Build the per-core Bass program. flags = (has_mask, has_b_in, has_b_qkv,
    has_b_o, has_ln_affine)."""
    import concourse.bass as bass
    import concourse.bacc as bacc
    import concourse.mybir as mybir
    import concourse.tile as tile
    from concourse.masks import make_identity

    has_mask, has_b_in, has_b_qkv, has_b_o, has_ln = flags
    f32 = mybir.dt.float32
    bf16 = mybir.dt.bfloat16
    i32 = mybir.dt.int32
    Alu = mybir.AluOpType
    Act = mybir.ActivationFunctionType

    NBLK = BC // P
    CH = C * HW  # 8192
    HNM = HEADS * N * N  # 100

    nc = bacc.Bacc("TRN2", target_bir_lowering=False, debug=False,
                   num_devices=NCORES)

    rf = nc.dram_tensor("rf", [BC, N, CH], f32, kind="ExternalInput")
    w_in_d = nc.dram_tensor("w_in_t", [4, P, HID], bf16, kind="ExternalInput")
    wqkv_d = nc.dram_tensor("wqkv_t", [LAYERS, 2, P, 3 * HID], bf16,
                            kind="ExternalInput")
    wo_d = nc.dram_tensor("wo_t", [LAYERS, 2, P, HID], bf16,
                          kind="ExternalInput")
    if has_b_in:
        b_in_d = nc.dram_tensor("b_in_t", [1, HID], bf16, kind="ExternalInput")
    if has_b_qkv:
        bqkv_d = nc.dram_tensor("bqkv_t", [LAYERS, 3 * HID], bf16,
                                kind="ExternalInput")
    if has_b_o:
        bo_d = nc.dram_tensor("bo_t", [LAYERS, HID], bf16, kind="ExternalInput")
    if has_ln:
        ln_g_d = nc.dram_tensor("ln_g_t", [LAYERS, HID], f32,
                                kind="ExternalInput")
        ln_b_d = nc.dram_tensor("ln_b_t", [LAYERS, HID], f32,
                                kind="ExternalInput")
    if has_mask:
        mask_d = nc.dram_tensor("mask_t", [N * N], f32, kind="ExternalInput")

    x_out = nc.dram_tensor("x_out", [BC, N, HID], f32, kind="ExternalOutput")
    attn_out = nc.dram_tensor("attn_out", [BC, HEADS * N * N], f32,
                              kind="ExternalOutput")

    def bcast_ap(base, off, dims):
        # custom AP over a tile: keep partition dim, replace free dims
        return bass.AP(tensor=base.tensor, offset=base.offset + off,
                       ap=[list(base.ap[0])] + [list(d) for d in dims])

    with tile.TileContext(nc) as tc:
        with (
            tc.tile_pool(name="consts", bufs=1) as consts,
            tc.tile_pool(name="xr", bufs=2) as xr_pool,
            tc.tile_pool(name="tree", bufs=1) as tree_pool,
            tc.tile_pool(name="pooled", bufs=2) as pooled_pool,
            tc.tile_pool(name="small", bufs=2) as small_pool,
            tc.tile_pool(name="xfer", bufs=6) as xfer_pool,
            tc.tile_pool(name="x", bufs=6) as x_pool,
            tc.tile_pool(name="qkv", bufs=3) as qkv_pool,
            # DVE-produced, DVE-consumed intermediates: bufs=1 (DVE is
            # serial anyway); cross-engine tiles get bufs>=2
            tc.tile_pool(name="attnd", bufs=1) as attnd_pool,
            tc.tile_pool(name="attnw", bufs=3) as attnw_pool,
            tc.tile_pool(name="stats", bufs=3) as stats_pool,
            tc.tile_pool(name="y", bufs=3) as y_pool,
            tc.tile_pool(name="psA", bufs=2, space="PSUM") as psA,
            tc.tile_pool(name="psT", bufs=1, space="PSUM") as psT,
            tc.tile_pool(name="psC", bufs=1, space="PSUM") as psC,
        ):
            # ---- constants ----
            ident32 = consts.tile([P, P], f32)
            make_identity(nc, ident32[:])
            identbf = consts.tile([P, P], bf16)
            make_identity(nc, identbf[:])
            magic_sb = consts.tile([P, N], i32)
            nc.vector.memset(magic_sb[:], 0x5F3759DF)

            w_in_sb = consts.tile([P, 4, HID], bf16)
            nc.sync.dma_start(out=w_in_sb[:],
                              in_=w_in_d[:].rearrange("k p o -> p k o"))
            wqkv_sb = consts.tile([P, LAYERS, 2, 3 * HID], bf16)
            nc.sync.dma_start(out=wqkv_sb[:],
                              in_=wqkv_d[:].rearrange("l k p o -> p l k o"))
            wo_sb = consts.tile([P, LAYERS, 2, HID], bf16)
            nc.sync.dma_start(out=wo_sb[:],
                              in_=wo_d[:].rearrange("l k p o -> p l k o"))

            if has_b_in or has_b_qkv or has_b_o:
                ones_row = consts.tile([1, P], bf16)
                nc.vector.memset(ones_row[:], 1.0)
            if has_b_in:
                b_in_sb = consts.tile([1, HID], bf16)
                nc.sync.dma_start(out=b_in_sb[:], in_=b_in_d[:])
            if has_b_qkv:
                bqkv_sb = consts.tile([1, LAYERS, 3 * HID], bf16)
                nc.sync.dma_start(out=bqkv_sb[:],
                                  in_=bqkv_d[:].rearrange("l o -> 1 l o"))
            if has_b_o:
                bo_sb = consts.tile([1, LAYERS, HID], bf16)
                nc.sync.dma_start(out=bo_sb[:],
                                  in_=bo_d[:].rearrange("l o -> 1 l o"))
            if has_ln:
                g_rep = consts.tile([P, LAYERS, HID], f32)
                nc.gpsimd.dma_start(
                    out=g_rep[:],
                    in_=bass.AP(tensor=ln_g_d, offset=0,
                                ap=[[0, P], [HID, LAYERS], [1, HID]]))
                b_rep = consts.tile([P, LAYERS, HID], f32)
                nc.gpsimd.dma_start(
                    out=b_rep[:],
                    in_=bass.AP(tensor=ln_b_d, offset=0,
                                ap=[[0, P], [HID, LAYERS], [1, HID]]))
            if has_mask:
                mask_rep = consts.tile([P, N * N], f32)
                nc.gpsimd.dma_start(
                    out=mask_rep[:],
                    in_=bass.AP(tensor=mask_d, offset=0,
                                ap=[[0, P], [1, N * N]]))

            xs = [None] * NBLK
            st = {}  # (l, blk) -> (y, sums, sumsq) handed from A to B

            def emit_p1(blk):
                """pooling + input projection -> xs[blk]"""
                b0 = blk * P
                x = x_pool.tile([P, N, HID], bf16, tag="x")
                xs[blk] = x
                for n in range(N):
                    xr = xr_pool.tile([P, C, HW], bf16)
                    rf_ap = rf[b0:b0 + P, n, :].rearrange("p (c h) -> p c h",
                                                          h=HW)
                    t1 = tree_pool.tile([P, C, 8], bf16, tag="t1")
                    if blk == 0 and n == 0:
                        # split the very first load 4-ways so the vector
                        # engine starts the pooling tree ~8us earlier
                        q = C // 4
                        for j in range(4):
                            nc.gpsimd.dma_start(out=xr[:, j * q:(j + 1) * q, :],
                                                in_=rf_ap[:, j * q:(j + 1) * q,
                                                          :])
                            nc.vector.tensor_add(
                                t1[:, j * q:(j + 1) * q, :],
                                xr[:, j * q:(j + 1) * q, 0:8],
                                xr[:, j * q:(j + 1) * q, 8:16])
                    else:
                        nc.gpsimd.dma_start(out=xr[:], in_=rf_ap)
                        nc.vector.tensor_add(t1[:], xr[:, :, 0:8],
                                             xr[:, :, 8:16])
                    t2 = tree_pool.tile([P, C, 4], bf16, tag="t2")
                    nc.vector.tensor_add(t2[:], t1[:, :, 0:4], t1[:, :, 4:8])
                    t3 = tree_pool.tile([P, C, 2], bf16, tag="t3")
                    nc.vector.tensor_add(t3[:], t2[:, :, 0:2], t2[:, :, 2:4])
                    pooled = pooled_pool.tile([P, C], f32, tag="pooled")
                    nc.vector.tensor_add(pooled[:], t3[:, :, 0], t3[:, :, 1])

                    pooledT_ps = psA.tile([P, 4, P], f32, tag="psA")
                    for j in range(4):
                        nc.tensor.transpose(pooledT_ps[:, j, :],
                                            pooled[:, j * P:(j + 1) * P],
                                            ident32[:])
                    pooledT = pooled_pool.tile([P, 4, P], bf16, tag="pooledT")
                    nc.scalar.copy(pooledT[:], pooledT_ps[:])

                    x_ps = psA.tile([P, HID], f32, tag="psA")
                    for j in range(4):
                        nc.tensor.matmul(x_ps[:], pooledT[:, j, :],
                                         w_in_sb[:, j, :], start=(j == 0),
                                         stop=(j == 3 and not has_b_in))
                    if has_b_in:
                        nc.tensor.matmul(x_ps[:], ones_row[:], b_in_sb[:],
                                         start=False, stop=True)
                    nc.scalar.copy(x[:, n, :], x_ps[:])

            def emit_A1(l, blk):
                """x transposes + QKV projection -> qkv"""
                x = xs[blk]
                qkv = qkv_pool.tile([P, N, 3, HID], bf16, tag="qkv")
                # transposes + evacuations first, then matmuls: keeps the
                # in-order ACT stream free of PE round-trip stalls
                xTs = []
                for n in range(N):
                    xT_ps = psT.tile([P, 2, P], bf16, tag="psT")
                    for j in range(2):
                        nc.tensor.transpose(xT_ps[:, j, :],
                                            x[:, n, j * P:(j + 1) * P],
                                            identbf[:])
                    xT = xfer_pool.tile([P, 2, P], bf16, tag="xT")
                    nc.scalar.copy(xT[:], xT_ps[:])
                    xTs.append(xT)
                for n in range(N):
                    xT = xTs[n]
                    qkv_ps = psA.tile([P, 3 * HID], f32, tag="psA")
                    for c0, c1 in ((0, 512), (512, 768)):
                        for j in range(2):
                            nc.tensor.matmul(qkv_ps[:, c0:c1], xT[:, j, :],
                                             wqkv_sb[:, l, j, c0:c1],
                                             start=(j == 0),
                                             stop=(j == 1 and not has_b_qkv))
                        if has_b_qkv:
                            nc.tensor.matmul(qkv_ps[:, c0:c1], ones_row[:],
                                             bqkv_sb[:, l, c0:c1],
                                             start=False, stop=True)
                    nc.scalar.copy(qkv[:, n, :, :], qkv_ps[:])
                st[("qkv", l, blk)] = qkv

            def emit_A2(l, blk):
                """attention (batch on partitions) -> ctx"""
                b0 = blk * P
                qkv = st.pop(("qkv", l, blk))
                # qkv free strides: n:768, slot:256, h:64, d:1
                qb = qkv[:]
                prod = attnd_pool.tile([P, HNM, HD], bf16, tag="bigprod")
                for h in range(HEADS):
                    # out block (n, m, d) at rows h*25..h*25+25
                    nc.vector.tensor_mul(
                        bcast_ap(prod[:], h * N * N * HD,
                                 [[N * HD, N], [HD, N], [1, HD]]),
                        bcast_ap(qb, h * HD,
                                 [[3 * HID, N], [0, N], [1, HD]]),
                        bcast_ap(qb, HID + h * HD,
                                 [[0, N], [3 * HID, N], [1, HD]]))
                s1 = attnd_pool.tile([P, HNM, 32], bf16, tag="s1")
                nc.vector.tensor_add(s1[:], prod[:, :, 0:32], prod[:, :, 32:64])
                s2 = attnd_pool.tile([P, HNM, 16], bf16, tag="s2")
                nc.vector.tensor_add(s2[:], s1[:, :, 0:16], s1[:, :, 16:32])
                s3 = attnd_pool.tile([P, HNM, 8], bf16, tag="s3")
                nc.vector.tensor_add(s3[:], s2[:, :, 0:8], s2[:, :, 8:16])
                s4 = attnd_pool.tile([P, HNM, 4], bf16, tag="s4")
                nc.vector.tensor_add(s4[:], s3[:, :, 0:4], s3[:, :, 4:8])
                s5 = attnd_pool.tile([P, HNM, 2], bf16, tag="s5")
                nc.vector.tensor_add(s5[:], s4[:, :, 0:2], s4[:, :, 2:4])
                scores = attnw_pool.tile([P, HEADS, N, N], bf16, tag="scores")
                nc.vector.tensor_add(
                    scores[:].rearrange("p h n m -> p (h n m)"),
                    s5[:, :, 0], s5[:, :, 1])
                if has_mask:
                    nc.vector.tensor_add(
                        scores[:], scores[:],
                        bcast_ap(mask_rep[:], 0, [[0, HEADS], [1, N * N]]))
                exps = attnw_pool.tile([P, HEADS, N, N], bf16, tag="exps")
                nc.scalar.activation(exps[:], scores[:], Act.Exp,
                                     scale=1.0 / np.sqrt(HD))
                Z = stats_pool.tile([P, HEADS * N], f32, tag="Z")
                nc.vector.tensor_reduce(
                    Z[:], exps[:].rearrange("p h n m -> p (h n) m"),
                    axis=mybir.AxisListType.X, op=Alu.add)
                rZ = stats_pool.tile([P, HEADS * N], f32, tag="rZ")
                nc.vector.reciprocal(rZ[:], Z[:])
                attnb = attnd_pool.tile([P, HEADS, N, N], bf16, tag="attnb")
                rz_ap = bcast_ap(rZ[:], 0, [[N, HEADS], [1, N], [0, N]])
                nc.vector.tensor_mul(attnb[:], exps[:], rz_ap)
                if l == LAYERS - 1:
                    attnf = attnw_pool.tile([P, HEADS, N, N], f32, tag="attnf")
                    nc.vector.tensor_mul(attnf[:], exps[:], rz_ap)
                    nc.sync.dma_start(
                        out=attn_out[b0:b0 + P, :],
                        in_=attnf[:].rearrange("p h n m -> p (h n m)"))

                # ctx[b, n, h, d] = sum_m attn[b,h,n,m] * V[b,m,h,d]
                # A2 = attn duplicated in d-pairs so products keep a step-1
                # even innermost dim (DVE 2x mode)
                A2 = attnd_pool.tile([P, HNM, 2], bf16, tag="A2")
                nc.vector.tensor_copy(
                    A2[:], bcast_ap(attnb[:], 0, [[1, HNM], [0, 2]]))
                # prod2 layout (n, h, m, d): strides n:1280, h:320, m:64, d:1
                prod2 = attnd_pool.tile([P, N * HEADS, N, HD], bf16,
                                        tag="bigprod")
                p2 = prod2[:]
                for h in range(HEADS):
                    for m in range(N):
                        nc.vector.tensor_mul(
                            bcast_ap(p2, h * N * HD + m * HD,
                                     [[HEADS * N * HD, N], [2, HD // 2],
                                      [1, 2]]),
                            bcast_ap(A2[:], (h * N * N + m) * 2,
                                     [[2 * N, N], [0, HD // 2], [1, 2]]),
                            bcast_ap(qb, 2 * HID + m * 3 * HID + h * HD,
                                     [[0, N], [2, HD // 2], [1, 2]]))
                # m-tree, d stays innermost -> all adds run 2x
                c1 = attnd_pool.tile([P, N * HEADS, 2, HD], bf16, tag="c1")
                nc.vector.tensor_add(
                    c1[:],
                    bcast_ap(p2, 0, [[N * HD, N * HEADS], [2 * HD, 2],
                                     [1, HD]]),
                    bcast_ap(p2, HD, [[N * HD, N * HEADS], [2 * HD, 2],
                                      [1, HD]]))
                c2 = attnd_pool.tile([P, N * HEADS, HD], bf16, tag="c2")
                nc.vector.tensor_add(c2[:], c1[:, :, 0, :], c1[:, :, 1, :])
                ctx = attnw_pool.tile([P, N, HID], bf16, tag="ctx")
                nc.vector.tensor_add(
                    ctx[:].rearrange("p n (h d) -> p (n h) d", d=HD),
                    c2[:],
                    bcast_ap(p2, 4 * HD, [[N * HD, N * HEADS], [1, HD]]))

                st[("ctx", l, blk)] = ctx

            def emit_A3(l, blk):
                """ctx transposes + O-projection + residual -> y"""
                x = xs[blk]
                ctx = st.pop(("ctx", l, blk))
                ctxTs = []
                for n in range(N):
                    ctxT_ps = psT.tile([P, 2, P], bf16, tag="psT")
                    for j in range(2):
                        nc.tensor.transpose(ctxT_ps[:, j, :],
                                            ctx[:, n, j * P:(j + 1) * P],
                                            identbf[:])
                    ctxT = xfer_pool.tile([P, 2, P], bf16, tag="ctxT")
                    nc.scalar.copy(ctxT[:], ctxT_ps[:])
                    ctxTs.append(ctxT)
                o_big = psC.tile([P, N, HID], f32, tag="psC")
                for n in range(N):
                    ctxT = ctxTs[n]
                    for j in range(2):
                        nc.tensor.matmul(o_big[:, n, :], ctxT[:, j, :],
                                         wo_sb[:, l, j, :],
                                         start=(j == 0), stop=False)
                    if has_b_o:
                        nc.tensor.matmul(o_big[:, n, :], ones_row[:],
                                         bo_sb[:, l, :], start=False,
                                         stop=False)
                    nc.tensor.matmul(o_big[:, n, :], identbf[:], x[:, n, :],
                                     start=False, stop=True)
                # one batched evacuation frees the PSUM slot early and lets
                # the LN tail read SBUF (faster DVE modes)
                y = y_pool.tile([P, N, HID], f32, tag="y")
                nc.scalar.copy(y[:], o_big[:])
                st[("y", l, blk)] = y

            def emit_B(l, blk):
                """LayerNorm tail -> xs[blk] (and final output DMA)"""
                b0 = blk * P
                y = st.pop(("y", l, blk))
                # per-token mean/variance on DVE
                mv = stats_pool.tile([P, N, 2], f32, tag="mv")
                for n in range(N):
                    bns = stats_pool.tile([P, 6], f32, tag="bns")
                    nc.vector.bn_stats(bns[:], y[:, n, :])
                    nc.vector.bn_aggr(mv[:, n, :], bns[:])
                var = stats_pool.tile([P, N], f32, tag="var")
                nc.vector.tensor_scalar_add(var[:], mv[:, :, 1], 1e-5)
                # rstd = 1/sqrt(var) on DVE: bit-trick + 3 Newton steps
                sh = stats_pool.tile([P, N], i32, tag="sh")
                nc.vector.tensor_scalar(sh[:], var[:].bitcast(i32), 1, None,
                                        op0=Alu.logical_shift_right)
                rstd = stats_pool.tile([P, N], f32, tag="rstd")
                nc.vector.tensor_sub(rstd[:].bitcast(i32), magic_sb[:], sh[:])
                for _ in range(2):
                    nt = stats_pool.tile([P, N], f32, tag="nt")
                    nc.vector.tensor_mul(nt[:], rstd[:], rstd[:])
                    nc.vector.tensor_mul(nt[:], nt[:], var[:])
                    nc.vector.tensor_scalar(nt[:], nt[:], -0.5, 1.5,
                                            op0=Alu.mult, op1=Alu.add)
                    nc.vector.tensor_mul(rstd[:], rstd[:], nt[:])

                final = l == LAYERS - 1
                if final:
                    # last layer's x goes straight to HBM: keep fp32
                    xn = x_pool.tile([P, N, HID], f32, tag="xf")
                else:
                    xn = x_pool.tile([P, N, HID], bf16, tag="x")
                # normalize on ScalarE: xn = y*rstd + (-mu*rstd)
                nmb = stats_pool.tile([P, N], f32, tag="nmb")
                nc.vector.scalar_tensor_tensor(nmb[:], mv[:, :, 0], -1.0,
                                               rstd[:], op0=Alu.mult,
                                               op1=Alu.mult)
                for n in range(N):
                    nc.scalar.activation(xn[:, n, :], y[:, n, :], Act.Identity,
                                         bias=nmb[:, n:n + 1],
                                         scale=rstd[:, n:n + 1])
                    if has_ln:
                        nc.vector.tensor_mul(xn[:, n, :], xn[:, n, :],
                                             g_rep[:, l, :])
                        nc.vector.tensor_add(xn[:, n, :], xn[:, n, :],
                                             b_rep[:, l, :])
                xs[blk] = xn
                if l == LAYERS - 1:
                    nc.sync.dma_start(out=x_out[b0:b0 + P, :, :], in_=xn[:])

            # ---- software-pipelined emission ----
            # Slot k handles A-stage (l,blk) = S[k]. Emitting next slot's QKV
            # (A1) before this slot's O-projection (A3), and LayerNorm (B)
            # two slots late, keeps every engine's in-order stream stall-free:
            # PE/ACT always have independent work while DVE runs attention.
            if NBLK == 1:
                emit_p1(0)
                for l in range(LAYERS):
                    emit_A1(l, 0)
                    emit_A2(l, 0)
                    emit_A3(l, 0)
                    emit_B(l, 0)
            else:
                S = [(l, blk) for l in range(LAYERS) for blk in range(NBLK)]
                emit_p1(0)
                emit_p1(1)
                for k in range(len(S)):
                    if k >= 2:
                        emit_B(*S[k - 2])
                    emit_A1(*S[k])
                    if k + 2 < NBLK:
                        emit_p1(k + 2)
                    if k >= 1:
                        emit_A3(*S[k - 1])
                    emit_A2(*S[k])
                emit_A3(*S[-1])
                emit_B(*S[-2])
                emit_B(*S[-1])

    nc.compile()
    return nc


def _get_program(flags, BC):
    key = (flags, BC)
    if key not in _BUILD_CACHE:
        _BUILD_CACHE[key] = _build(flags, BC)
    return _BUILD_CACHE[key]


def kernel(region_features, adjacency, w_in, b_in, wq, bq, wk, bk, wv, bv,
           wo, bo, ln_g, ln_b, _trace=False, _bc=None):
    from concourse.bass_utils import run_bass_kernel_spmd

    region_features = np.asarray(region_features)
    adjacency = np.asarray(adjacency, dtype=np.float32)
    w_in = np.asarray(w_in, dtype=np.float32)
    b_in = np.asarray(b_in, dtype=np.float32)
    wq, wk, wv, wo = (np.asarray(a, dtype=np.float32) for a in (wq, wk, wv, wo))
    bq, bk, bv, bo = (np.asarray(a, dtype=np.float32) for a in (bq, bk, bv, bo))
    ln_g = np.asarray(ln_g, dtype=np.float32)
    ln_b = np.asarray(ln_b, dtype=np.float32)

    Btot = region_features.shape[0]
    BC = _bc if _bc is not None else Btot // NCORES
    assert Btot == BC * NCORES

    has_mask = bool((adjacency == 0).any())
    has_b_in = bool(np.any(b_in != 0))
    has_b_qkv = bool(np.any(bq != 0) or np.any(bk != 0) or np.any(bv != 0))
    has_b_o = bool(np.any(bo != 0))
    has_ln = bool(np.any(ln_g != 1) or np.any(ln_b != 0))
    flags = (has_mask, has_b_in, has_b_qkv, has_b_o, has_ln)

    nc = _get_program(flags, BC)

    bf = ml_dtypes.bfloat16
    # host-side weight prep: fold /16 pooling into w_in, chunk contraction dim
    w_in_t = np.ascontiguousarray((w_in / HW).reshape(4, P, HID)).astype(bf)
    wqkv = np.concatenate([wq, wk, wv], axis=2)  # [L, 256, 768]
    wqkv_t = np.ascontiguousarray(wqkv.reshape(LAYERS, 2, P, 3 * HID)).astype(bf)
    wo_t = np.ascontiguousarray(wo.reshape(LAYERS, 2, P, HID)).astype(bf)

    base_map = {
        "w_in_t": w_in_t,
        "wqkv_t": wqkv_t,
        "wo_t": wo_t,
    }
    if has_b_in:
        base_map["b_in_t"] = b_in.reshape(1, HID).astype(bf)
    if has_b_qkv:
        base_map["bqkv_t"] = np.concatenate([bq, bk, bv], axis=1).astype(bf)
    if has_b_o:
        base_map["bo_t"] = bo.astype(bf)
    if has_ln:
        base_map["ln_g_t"] = ln_g
        base_map["ln_b_t"] = ln_b
    if has_mask:
        base_map["mask_t"] = np.where(adjacency == 0, -1e9,
                                      0.0).astype(np.float32).reshape(N * N)

    rf_flat = np.ascontiguousarray(
        region_features.reshape(Btot, N, C * HW)).astype(np.float32)
    in_maps = []
    for c in range(NCORES):
        m = dict(base_map)
        m["rf"] = rf_flat[c * BC:(c + 1) * BC]
        in_maps.append(m)

    res = run_bass_kernel_spmd(nc, in_maps, core_ids=list(range(NCORES)),
                               trace=_trace)
    kernel.last_results = res

    x_full = np.concatenate([r["x_out"] for r in res.results], axis=0)
    attn_full = np.concatenate(
        [r["attn_out"].reshape(BC, HEADS, N, N) for r in res.results], axis=0)
    return x_full, attn_full
